# revision 1
# baseline (speedup 1.0000x reference)
"""Full-attention kernel (QKV projections + softmax(QK^T/sqrt(d))V) on 8
trn2 NeuronCores.

Problem: x [2,4096,512] f32, W_{q,k,v} [512,512] f32 -> context [2,4096,512]
f32 (the reference applies no causal mask and dropout=0).

Distribution (data parallel, no collectives -- measured faster than
AllGather-sharded projections on this fabric): core c handles batch
b = c // 4 and query block q0 = (c % 4) * 1024; each core redundantly
projects K^T/V for its whole batch in 4 streamed key passes.  The host
rotates each core's copy of x[b] so its query rows come first (attention
is permutation-invariant over keys, so key order is irrelevant).

Precision: float32r matmuls and PE-transposes (1/1.5 cyc/row), fp32 PSUM
accumulation, fp32 softmax stats; exp on ACT with fused row-sum
(accum_out); no max-subtraction (scores are O(5) by construction);
context normalized by 1/rowsum per query tile as soon as its last key
pass completes.
"""
import numpy as np
from contextlib import ExitStack

from concourse import bacc
import concourse.mybir as mybir
import concourse.tile as tile
from concourse.bass_utils import run_bass_kernel_spmd
from concourse.masks import make_identity

F32 = mybir.dt.float32
F32R = mybir.dt.float32r
BF16 = mybir.dt.bfloat16
AF = mybir.ActivationFunctionType
ADD = mybir.AluOpType.add
AX = mybir.AxisListType

B, S, D = 2, 4096, 512
N_CORES = 8
CORES_PER_B = N_CORES // B
QLEN = S // CORES_PER_B             # 1024
NPASS = 4
KLEN = S // NPASS                   # 1024
P = 128
SCALE = 1.0 / float(np.sqrt(D))

N_QT = QLEN // P                    # 8
N_KB = KLEN // 512                  # 2
N_ST = KLEN // P                    # 8
N_DC = D // P                       # 4


def _build(nreps=1):
    nc = bacc.Bacc(None)
    xb_d = nc.declare_dram_parameter("xb", [S, D], F32R, isOutput=False)
    wq_d = nc.declare_dram_parameter("wq", [D, D], F32R, isOutput=False)
    wk_d = nc.declare_dram_parameter("wk", [D, D], F32R, isOutput=False)
    wv_d = nc.declare_dram_parameter("wv", [D, D], F32R, isOutput=False)
    out_d = nc.declare_dram_parameter("out", [QLEN, D], F32, isOutput=True)

    with tile.TileContext(nc) as tc, ExitStack() as ctx:
        const = ctx.enter_context(tc.tile_pool(name="const", bufs=1))
        w_pool = ctx.enter_context(tc.tile_pool(name="w", bufs=1))
        x_pool = ctx.enter_context(tc.tile_pool(name="x", bufs=8))
        xT_pool = ctx.enter_context(tc.tile_pool(name="xT", bufs=2))
        kT_pool = ctx.enter_context(tc.tile_pool(name="kT", bufs=2))
        v_pool = ctx.enter_context(tc.tile_pool(name="v", bufs=2))
        qT_pool = ctx.enter_context(tc.tile_pool(name="qT", bufs=1))
        pr_pool = ctx.enter_context(tc.tile_pool(name="pr", bufs=4))
        prT_pool = ctx.enter_context(tc.tile_pool(name="prT", bufs=4))
        acc_pool = ctx.enter_context(tc.tile_pool(name="acc", bufs=1))
        st_pool = ctx.enter_context(tc.tile_pool(name="st", bufs=1))

        ps_tr = ctx.enter_context(tc.tile_pool(name="ps_tr", bufs=2, space="PSUM"))
        ps_pj = ctx.enter_context(tc.tile_pool(name="ps_pj", bufs=2, space="PSUM"))
        ps_sc = ctx.enter_context(tc.tile_pool(name="ps_sc", bufs=2, space="PSUM"))
        ps_cx = ctx.enter_context(tc.tile_pool(name="ps_cx", bufs=2, space="PSUM"))

        ident_f = const.tile([P, P], F32)
        make_identity(nc, ident_f[:])
        ident = const.tile([P, P], F32R)
        nc.vector.tensor_copy(ident[:], ident_f[:])
        zbias = const.tile([P, 1], F32)
        nc.vector.memset(zbias[:], 0.0)

        # warm the PE/HAM clock gate with dummy transposes while the first
        # x tiles are still in flight on the DMA queues
        warm = ps_tr.tile([P, N_DC, P], F32R, tag="tr", name="warm")
        for _w in range(16):
            nc.tensor.matmul(warm[:, _w % N_DC, :], ident[:], ident[:],
                             is_transpose=True, start=True, stop=True)

        acc = acc_pool.tile([P, N_QT, D], F32)
        rsums = st_pool.tile([P, N_QT, NPASS * N_KB], F32)
        rtot = st_pool.tile([P, N_QT], F32)
        recip = st_pool.tile([P, N_QT], F32)

        w_tiles = {}

        def emit_W():
            # gpsimd(SWDGE)-issued DMAs: keep SP/ACT queues free for x tiles
            for name, wd in (("wk", wk_d), ("wv", wv_d), ("wq", wq_d)):
                wt = w_pool.tile([P, N_DC, D], F32R, tag=name)
                for c in range(N_DC):
                    nc.gpsimd.dma_start(out=wt[:, c, :],
                                        in_=wd[c * P:(c + 1) * P, :])
                w_tiles[name] = wt

        qT = {}
        kT = {}
        v = {}

        def emit_A(p, first=False):
            r0 = p * KLEN
            xT = xT_pool.tile([P, N_DC, KLEN], F32R, tag="xT")
            for st in range(N_ST):
                x_t = x_pool.tile([P, D], F32R, tag="x")
                xeng = nc.sync if st % 2 == 0 else nc.scalar
                if first and st < 2:
                    # column-chunk loads so the first transposes start sooner
                    for c in range(N_DC):
                        xeng.dma_start(
                            out=x_t[:, c * P:(c + 1) * P],
                            in_=xb_d[r0 + st * P:r0 + (st + 1) * P,
                                     c * P:(c + 1) * P])
                else:
                    xeng.dma_start(
                        out=x_t[:], in_=xb_d[r0 + st * P:r0 + (st + 1) * P, :])
                if first and st == 0:
                    emit_W()
                ptr = ps_tr.tile([P, N_DC, P], F32R, tag="tr")
                for c in range(N_DC):
                    nc.tensor.matmul(
                        ptr[:, c, :], x_t[:, c * P:(c + 1) * P], ident[:],
                        is_transpose=True, start=True, stop=True)
                nc.scalar.copy(xT[:, :, st * P:(st + 1) * P], ptr[:])
            wq_t, wk_t, wv_t = w_tiles["wq"], w_tiles["wk"], w_tiles["wv"]
            kt = kT_pool.tile([P, N_DC, KLEN], F32R, tag="kT")
            for do in range(N_DC):
                for blk in range(KLEN // 512):
                    pp = ps_pj.tile([P, 512], F32, tag="pj")
                    for c in range(N_DC):
                        nc.tensor.matmul(
                            pp[:], wk_t[:, c, do * P:(do + 1) * P],
                            xT[:, c, blk * 512:(blk + 1) * 512],
                            start=(c == 0), stop=(c == N_DC - 1))
                    nc.vector.tensor_copy(kt[:, do, blk * 512:(blk + 1) * 512],
                                          pp[:])
            vt = v_pool.tile([P, N_ST, D], F32R, tag="v")
            for st in range(N_ST):
                pp = ps_pj.tile([P, 512], F32, tag="pj")
                for c in range(N_DC):
                    nc.tensor.matmul(
                        pp[:], xT[:, c, st * P:(st + 1) * P], wv_t[:, c, :],
                        start=(c == 0), stop=(c == N_DC - 1))
                nc.vector.tensor_copy(vt[:, st, :], pp[:])
            if p == 0:
                qt_ = qT_pool.tile([P, N_DC, QLEN], F32R, tag="qT")
                for do in range(N_DC):
                    for blk in range(QLEN // 512):
                        pp = ps_pj.tile([P, 512], F32, tag="pj")
                        for c in range(N_DC):
                            nc.tensor.matmul(
                                pp[:], wq_t[:, c, do * P:(do + 1) * P],
                                xT[:, c, blk * 512:(blk + 1) * 512],
                                start=(c == 0), stop=(c == N_DC - 1))
                        nc.scalar.mul(qt_[:, do, blk * 512:(blk + 1) * 512],
                                      pp[:], SCALE)
                qT["t"] = qt_
            kT[p] = kt
            v[p] = vt

        def finalize(qt):
            nc.vector.tensor_reduce(rtot[:, qt:qt + 1], rsums[:, qt, :],
                                    axis=AX.X, op=ADD)
            nc.vector.reciprocal(recip[:, qt:qt + 1], rtot[:, qt:qt + 1])
            # halves on separate DMA queues so the tail store pipelines
            for h, eng in ((0, nc.sync), (1, nc.scalar)):
                cols = slice(h * (D // 2), (h + 1) * (D // 2))
                nc.vector.tensor_scalar_mul(acc[:, qt, cols], acc[:, qt, cols],
                                            recip[:, qt:qt + 1])
                eng.dma_start(out=out_d[qt * P:(qt + 1) * P, cols],
                              in_=acc[:, qt, cols])

        def emit_B(p):
            kt, vt, qt_ = kT[p], v[p], qT["t"]
            for qt in range(N_QT):
                pcx = ps_cx.tile([P, D], F32, tag="cx")
                n_mm = N_KB * N_DC
                mm = 0
                for kb in range(N_KB):
                    psc = ps_sc.tile([P, 512], F32, tag="sc")
                    for c in range(N_DC):
                        nc.tensor.matmul(
                            psc[:], qt_[:, c, qt * P:(qt + 1) * P],
                            kt[:, c, kb * 512:(kb + 1) * 512],
                            start=(c == 0), stop=(c == N_DC - 1))
                    probs = pr_pool.tile([P, 512], F32R, tag="pr")
                    nc.scalar.activation(
                        probs[:], psc[:], AF.Exp, bias=zbias[:],
                        accum_out=rsums[:, qt, p * N_KB + kb:p * N_KB + kb + 1])
                    ptr = ps_tr.tile([P, N_DC, P], F32R, tag="tr")
                    for j in range(N_DC):
                        nc.tensor.matmul(
                            ptr[:, j, :], probs[:, j * P:(j + 1) * P], ident[:],
                            is_transpose=True, start=True, stop=True)
                    prT = prT_pool.tile([P, N_DC, P], F32R, tag="prT")
                    nc.vector.tensor_copy(prT[:], ptr[:])
                    for j in range(N_DC):
                        nc.tensor.matmul(
                            pcx[:], prT[:, j, :], vt[:, kb * N_DC + j, :],
                            start=(mm == 0), stop=(mm == n_mm - 1))
                        mm += 1
                if p == 0:
                    nc.vector.tensor_copy(acc[:, qt, :], pcx[:])
                else:
                    nc.vector.tensor_tensor(acc[:, qt, :], pcx[:],
                                            acc[:, qt, :], op=ADD)
                if p == NPASS - 1:
                    finalize(qt)

        for _rep in range(nreps):
            emit_A(0, first=(_rep == 0))
            emit_A(1)
            emit_B(0)
            emit_A(2)
            emit_B(1)
            emit_A(3)
            emit_B(2)
            emit_B(3)

    nc.compile()
    return nc


_CACHE = {}


def _get_nc(nreps=1):
    if nreps not in _CACHE:
        _CACHE[nreps] = _build(nreps)
    return _CACHE[nreps]


def _in_maps(x, W_query, W_key, W_value):
    x = np.ascontiguousarray(np.asarray(x, dtype=np.float32))
    wq = np.ascontiguousarray(np.asarray(W_query, dtype=np.float32))
    wk = np.ascontiguousarray(np.asarray(W_key, dtype=np.float32))
    wv = np.ascontiguousarray(np.asarray(W_value, dtype=np.float32))
    maps = []
    for core in range(N_CORES):
        b = core // CORES_PER_B
        q0 = (core % CORES_PER_B) * QLEN
        xb = np.roll(x[b], -q0, axis=0)
        maps.append({"xb": xb, "wq": wq, "wk": wk, "wv": wv})
    return maps


def kernel(x, W_query, W_key, W_value, _trace=False):
    import os
    if not _trace:
        # NTFF tracing is unavailable here; make sure an inherited
        # BASS_TRACE can't route execution down that path.
        os.environ.setdefault("BASS_NEVER_TRACE", "1")
    nc = _get_nc()
    maps = _in_maps(x, W_query, W_key, W_value)
    res = run_bass_kernel_spmd(nc, maps, list(range(N_CORES)), trace=_trace)
    out = np.empty((B, S, D), dtype=np.float32)
    for core in range(N_CORES):
        b = core // CORES_PER_B
        q0 = (core % CORES_PER_B) * QLEN
        out[b, q0:q0 + QLEN] = res.results[core]["out"]
    if _trace:
        return out, res
    return out



# revision 5
# speedup vs baseline: 1.1608x; 1.1608x over previous
"""Full-attention kernel (QKV projections + softmax(QK^T/sqrt(d))V) on 8
trn2 NeuronCores.

Problem: x [2,4096,512] f32, W_{q,k,v} [512,512] f32 -> context [2,4096,512]
f32 (the reference applies no causal mask and dropout=0).

Distribution (data parallel, no collectives): core c handles batch b = c // 4
and query block q0 = (c % 4) * 1024; each core redundantly projects K^T/V for
its whole batch.  The host rotates each core's copy of x[b] so its query rows
come first (attention is permutation-invariant over keys) and ships x
pre-transposed as x^T [D, S] in fp16, so the kernel needs no PE transposes at
all.

Layout/precision choices (all driven by the PE cost model: 1 cyc/output-row
for fp16, f32 psum accumulation):
  * scores are computed TRANSPOSED, [k_tile=128, q=256], with lhsT = K^T
    chunks and rhs = Q^T chunks; exp(scores^T) on ACT then feeds the PV
    matmul directly as stationary (lhsT) -- no probability transposes.
  * the softmax row-sum is folded into the PV matmul: V is stored as
    [V[:, :256] | ones] (257 cols) + V[:, 256:] (256 cols), so each query's
    sum(exp) accumulates in psum column 256 -- landing per-partition,
    exactly where the 1/rowsum normalization needs it.  Row-sum cost: one
    extra psum column per PV matmul (256 PE cycles total).
  * fp16 (not bf16): same 1.0 cyc/row on PE, 8 extra mantissa bits.
"""
import numpy as np
from contextlib import ExitStack

from concourse import bacc
import concourse.mybir as mybir
import concourse.tile as tile
from concourse.bass_utils import run_bass_kernel_spmd
from concourse.masks import make_identity

F32 = mybir.dt.float32
F32R = mybir.dt.float32r
F16 = mybir.dt.float16
AF = mybir.ActivationFunctionType

B, S, D = 2, 4096, 512
N_CORES = 8
CORES_PER_B = N_CORES // B
QLEN = S // CORES_PER_B             # 1024
P = 128
SCALE = 1.0 / float(np.sqrt(D))

N_DC = D // P                       # 4 contraction chunks
N_KT = S // P                       # 32 key tiles
N_KB = S // 512                     # 8 key blocks (projection granularity)
QB = 256                            # query block (psum-bank friendly)
N_QB = QLEN // QB                   # 4


def _build(nreps=1):
    nc = bacc.Bacc(None)
    xT_d = nc.declare_dram_parameter("xT", [D, S], F16, isOutput=False)
    wq_d = nc.declare_dram_parameter("wq", [D, D], F16, isOutput=False)
    wk_d = nc.declare_dram_parameter("wk", [D, D], F16, isOutput=False)
    wv_d = nc.declare_dram_parameter("wv", [D, D], F16, isOutput=False)
    out_d = nc.declare_dram_parameter("out", [QLEN, D], F32, isOutput=True)

    with tile.TileContext(nc) as tc, ExitStack() as ctx:
        const = ctx.enter_context(tc.tile_pool(name="const", bufs=1))
        big = ctx.enter_context(tc.tile_pool(name="big", bufs=1))
        pr_pool = ctx.enter_context(tc.tile_pool(name="pr", bufs=4))
        st_pool = ctx.enter_context(tc.tile_pool(name="st", bufs=2))
        # psum: "a" ring (proj + scores) 4 banks, warm 1 bank, cx 3 banks
        ps_a = ctx.enter_context(tc.tile_pool(name="ps_a", bufs=4, space="PSUM"))
        ps_cx = ctx.enter_context(tc.tile_pool(name="ps_cx", bufs=1, space="PSUM"))

        ident_f = const.tile([P, P], F32)
        make_identity(nc, ident_f[:])
        ident = const.tile([P, P], F32R)
        nc.vector.tensor_copy(ident[:], ident_f[:])

        # warm the PE clock gate with dummy transposes while DMAs stream in
        # (reuses the "a" psum ring so no extra bank is consumed)
        for _w in range(16):
            warm = ps_a.tile([P, N_DC, P], F32R, tag="a", name="warm")
            nc.tensor.matmul(warm[:, _w % N_DC, :], ident[:], ident[:],
                             is_transpose=True, start=True, stop=True)

        # ---- input DMAs -------------------------------------------------
        wq_t = big.tile([P, N_DC, D], F16, tag="wq")
        wk_t = big.tile([P, N_DC, D], F16, tag="wk")
        wv_t = big.tile([P, N_DC, D], F16, tag="wv")
        xT = big.tile([P, N_DC, S], F16, tag="xT")

        # DMA queue plan (only SP/ACT/gpsimd can issue):
        #   sync:   wq c0,c1 | xT c0,c1 query cols | xT c0,c1 rest
        #   scalar: wq c2,c3 | xT c2,c3 query cols | wk | xT c2,c3 rest
        #   gpsimd: wv
        # Q projection can start ~4us in; K/V weights land well before use.
        x_engs = (nc.sync, nc.sync, nc.scalar, nc.scalar)
        nc.sync.dma_start(out=wq_t[:, 0, :], in_=wq_d[0:P, :])
        nc.sync.dma_start(out=wq_t[:, 1, :], in_=wq_d[P:2 * P, :])
        nc.scalar.dma_start(out=wq_t[:, 2, :], in_=wq_d[2 * P:3 * P, :])
        nc.scalar.dma_start(out=wq_t[:, 3, :], in_=wq_d[3 * P:4 * P, :])
        for c in range(N_DC):
            nc.gpsimd.dma_start(out=wv_t[:, c, :], in_=wv_d[c * P:(c + 1) * P, :])
        for c in (0, 2, 1, 3):
            x_engs[c].dma_start(out=xT[:, c, 0:QLEN],
                                in_=xT_d[c * P:(c + 1) * P, 0:QLEN])
        for c in range(N_DC):
            nc.scalar.dma_start(out=wk_t[:, c, :], in_=wk_d[c * P:(c + 1) * P, :])
        for c in (0, 2, 1, 3):
            x_engs[c].dma_start(out=xT[:, c, QLEN:S],
                                in_=xT_d[c * P:(c + 1) * P, QLEN:S])

        # ---- projections ------------------------------------------------
        qT = big.tile([P, N_DC, QLEN], F16, tag="qT")
        for do in range(N_DC):
            for qh in range(QLEN // 512):
                pp = ps_a.tile([P, 512], F32, tag="a", name="pp")
                for c in range(N_DC):
                    nc.tensor.matmul(
                        pp[:], wq_t[:, c, do * P:(do + 1) * P],
                        xT[:, c, qh * 512:(qh + 1) * 512],
                        start=(c == 0), stop=(c == N_DC - 1))
                nc.scalar.copy(qT[:, do, qh * 512:(qh + 1) * 512], pp[:])

        kT = big.tile([P, N_DC, S], F16, tag="kT")
        vtA = big.tile([P, N_KT, 257], F16, tag="vtA")
        vtB = big.tile([P, N_KT, 256], F16, tag="vtB")
        nc.vector.memset(vtA[:, :, 256:257], 1.0)

        for blk in range(N_KB):
            k0 = blk * 512
            for do in range(N_DC):
                pp = ps_a.tile([P, 512], F32, tag="a", name="pp")
                for c in range(N_DC):
                    nc.tensor.matmul(
                        pp[:], wk_t[:, c, do * P:(do + 1) * P],
                        xT[:, c, k0:k0 + 512],
                        start=(c == 0), stop=(c == N_DC - 1))
                nc.vector.tensor_copy(kT[:, do, k0:k0 + 512], pp[:])
            for j in range(4):
                kt = blk * 4 + j
                pp = ps_a.tile([P, 512], F32, tag="a", name="pp")
                for c in range(N_DC):
                    nc.tensor.matmul(
                        pp[:], xT[:, c, kt * P:(kt + 1) * P], wv_t[:, c, :],
                        start=(c == 0), stop=(c == N_DC - 1))
                nc.vector.tensor_copy(vtA[:, kt, 0:256], pp[:, 0:256])
                nc.vector.tensor_copy(vtB[:, kt, :], pp[:, 256:512])

        # ---- attention --------------------------------------------------
        st_engs = (nc.sync, nc.gpsimd)
        for qb in range(N_QB):
            q0 = qb * QB
            cxA = ps_cx.tile([P, 257], F32, tag="cxA", name="cxA")
            cxB = ps_cx.tile([P, 257], F32, tag="cxB", name="cxB")
            cxP0 = ps_cx.tile([P, 256], F32, tag="cxP0", name="cxP0")
            cxP1 = ps_cx.tile([P, 256], F32, tag="cxP1", name="cxP1")
            probs = {}

            def emit_sc(kt):
                psc = ps_a.tile([P, QB], F32, tag="a", name="psc")
                for c in range(N_DC):
                    nc.tensor.matmul(
                        psc[:], kT[:, c, kt * P:(kt + 1) * P],
                        qT[:, c, q0:q0 + QB],
                        start=(c == 0), stop=(c == N_DC - 1))
                pr = pr_pool.tile([P, QB], F16, tag="pr", name="pr")
                nc.scalar.activation(pr[:], psc[:], AF.Exp)
                probs[kt] = pr

            def emit_pv(kt):
                pr = probs.pop(kt)
                first = kt == 0
                last = kt == N_KT - 1
                nc.tensor.matmul(cxA[:], pr[:, 0:P], vtA[:, kt, :],
                                 start=first, stop=last)
                nc.tensor.matmul(cxP0[:], pr[:, 0:P], vtB[:, kt, :],
                                 start=first, stop=last)
                nc.tensor.matmul(cxB[:], pr[:, P:QB], vtA[:, kt, :],
                                 start=first, stop=last)
                nc.tensor.matmul(cxP1[:], pr[:, P:QB], vtB[:, kt, :],
                                 start=first, stop=last)

            emit_sc(0)
            emit_sc(1)
            for kt in range(N_KT):
                if kt + 2 < N_KT:
                    emit_sc(kt + 2)
                emit_pv(kt)

            for h, (cx, cxp) in enumerate(((cxA, cxP0), (cxB, cxP1))):
                qrow = q0 + h * P
                recip = st_pool.tile([P, 1], F32, tag="recip", name="recip")
                nc.vector.reciprocal(recip[:], cx[:, 256:257])
                osb = st_pool.tile([P, D], F32, tag="osb", name="osb")
                nc.vector.tensor_scalar_mul(osb[:, 0:256], cx[:, 0:256],
                                            recip[:])
                nc.vector.tensor_scalar_mul(osb[:, 256:512], cxp[:],
                                            recip[:])
                st_engs[h].dma_start(out=out_d[qrow:qrow + P, :], in_=osb[:])

    nc.compile()
    return nc


_CACHE = {}


def _get_nc(nreps=1):
    if nreps not in _CACHE:
        _CACHE[nreps] = _build(nreps)
    return _CACHE[nreps]


def _in_maps(x, W_query, W_key, W_value):
    x = np.asarray(x, dtype=np.float32)
    wq = np.ascontiguousarray(
        np.asarray(W_query, dtype=np.float32) * SCALE).astype(np.float16)
    wk = np.ascontiguousarray(np.asarray(W_key, dtype=np.float32)).astype(
        np.float16)
    wv = np.ascontiguousarray(np.asarray(W_value, dtype=np.float32)).astype(
        np.float16)
    maps = []
    for core in range(N_CORES):
        b = core // CORES_PER_B
        q0 = (core % CORES_PER_B) * QLEN
        xT = np.ascontiguousarray(
            np.roll(x[b], -q0, axis=0).T.astype(np.float16))
        maps.append({"xT": xT, "wq": wq, "wk": wk, "wv": wv})
    return maps


def kernel(x, W_query, W_key, W_value, _trace=False):
    import os
    if not _trace:
        # NTFF tracing is unavailable here; make sure an inherited
        # BASS_TRACE can't route execution down that path.
        os.environ.setdefault("BASS_NEVER_TRACE", "1")
    nc = _get_nc()
    maps = _in_maps(x, W_query, W_key, W_value)
    res = run_bass_kernel_spmd(nc, maps, list(range(N_CORES)), trace=_trace)
    out = np.empty((B, S, D), dtype=np.float32)
    for core in range(N_CORES):
        b = core // CORES_PER_B
        q0 = (core % CORES_PER_B) * QLEN
        out[b, q0:q0 + QLEN] = res.results[core]["out"]
    if _trace:
        return out, res
    return out


# revision 6
# speedup vs baseline: 1.2053x; 1.0384x over previous
"""Full-attention kernel (QKV projections + softmax(QK^T/sqrt(d))V) on 8
trn2 NeuronCores.

Problem: x [2,4096,512] f32, W_{q,k,v} [512,512] f32 -> context [2,4096,512]
f32 (the reference applies no causal mask and dropout=0).

Distribution (data parallel, no collectives): core c handles batch b = c // 4
and query block q0 = (c % 4) * 1024; each core redundantly projects K^T/V for
its whole batch.  The host rotates each core's copy of x[b] so its query rows
come first (attention is permutation-invariant over keys) and ships x
pre-transposed as x^T [D, S] in fp16, so the kernel needs no PE transposes at
all.

Layout/precision choices (all driven by the PE cost model: 1 cyc/output-row
for fp16, f32 psum accumulation):
  * scores are computed TRANSPOSED, [k_tile=128, q=256], with lhsT = K^T
    chunks and rhs = Q^T chunks; exp(scores^T) on ACT then feeds the PV
    matmul directly as stationary (lhsT) -- no probability transposes.
  * the softmax row-sum is folded into the PV matmul: V is stored as
    [V[:, :256] | ones] (257 cols) + V[:, 256:] (256 cols), so each query's
    sum(exp) accumulates in psum column 256 -- landing per-partition,
    exactly where the 1/rowsum normalization needs it.  Row-sum cost: one
    extra psum column per PV matmul (256 PE cycles total).
  * fp16 (not bf16): same 1.0 cyc/row on PE, 8 extra mantissa bits.
"""
import numpy as np
from contextlib import ExitStack

from concourse import bacc
import concourse.mybir as mybir
import concourse.tile as tile
from concourse.bass_utils import run_bass_kernel_spmd
from concourse.masks import make_identity

F32 = mybir.dt.float32
F32R = mybir.dt.float32r
F16 = mybir.dt.float16
AF = mybir.ActivationFunctionType

B, S, D = 2, 4096, 512
N_CORES = 8
CORES_PER_B = N_CORES // B
QLEN = S // CORES_PER_B             # 1024
P = 128
SCALE = 1.0 / float(np.sqrt(D))

N_DC = D // P                       # 4 contraction chunks
N_KT = S // P                       # 32 key tiles
N_KB = S // 512                     # 8 key blocks (projection granularity)
QB = 256                            # query block (psum-bank friendly)
N_QB = QLEN // QB                   # 4


def _build(nreps=1):
    nc = bacc.Bacc(None)
    xT_d = nc.declare_dram_parameter("xT", [D, S], F16, isOutput=False)
    wq_d = nc.declare_dram_parameter("wq", [D, D], F16, isOutput=False)
    wk_d = nc.declare_dram_parameter("wk", [D, D], F16, isOutput=False)
    wv_d = nc.declare_dram_parameter("wv", [D, D], F16, isOutput=False)
    out_d = nc.declare_dram_parameter("out", [QLEN, D], F32, isOutput=True)

    with tile.TileContext(nc) as tc, ExitStack() as ctx:
        const = ctx.enter_context(tc.tile_pool(name="const", bufs=1))
        big = ctx.enter_context(tc.tile_pool(name="big", bufs=1))
        pr_pool = ctx.enter_context(tc.tile_pool(name="pr", bufs=4))
        st_pool = ctx.enter_context(tc.tile_pool(name="st", bufs=2))
        # psum: "a" ring (proj + scores) 4 banks, warm 1 bank, cx 3 banks
        ps_a = ctx.enter_context(tc.tile_pool(name="ps_a", bufs=4, space="PSUM"))
        ps_cx = ctx.enter_context(tc.tile_pool(name="ps_cx", bufs=1, space="PSUM"))

        ident_f = const.tile([P, P], F32)
        make_identity(nc, ident_f[:])
        ident = const.tile([P, P], F32R)
        nc.vector.tensor_copy(ident[:], ident_f[:])

        # warm the PE clock gate with dummy transposes while DMAs stream in
        # (reuses the "a" psum ring so no extra bank is consumed)
        for _w in range(16):
            warm = ps_a.tile([P, N_DC, P], F32R, tag="a", name="warm")
            nc.tensor.matmul(warm[:, _w % N_DC, :], ident[:], ident[:],
                             is_transpose=True, start=True, stop=True)

        # ---- input DMAs -------------------------------------------------
        wq_t = big.tile([P, N_DC, D], F16, tag="wq")
        wk_t = big.tile([P, N_DC, D], F16, tag="wk")
        wv_t = big.tile([P, N_DC, D], F16, tag="wv")
        xT = big.tile([P, N_DC, S], F16, tag="xT")

        # DMA queue plan (only SP/ACT/gpsimd can issue):
        #   sync:   wq c0,c1 | xT c0,c1 query cols | xT c0,c1 rest
        #   scalar: wq c2,c3 | xT c2,c3 query cols | wk | xT c2,c3 rest
        #   gpsimd: wv
        # Q projection can start ~4us in; K/V weights land well before use.
        x_engs = (nc.sync, nc.sync, nc.scalar, nc.scalar)
        nc.sync.dma_start(out=wq_t[:, 0, :], in_=wq_d[0:P, :])
        nc.sync.dma_start(out=wq_t[:, 1, :], in_=wq_d[P:2 * P, :])
        nc.scalar.dma_start(out=wq_t[:, 2, :], in_=wq_d[2 * P:3 * P, :])
        nc.scalar.dma_start(out=wq_t[:, 3, :], in_=wq_d[3 * P:4 * P, :])
        for c in range(N_DC):
            nc.gpsimd.dma_start(out=wv_t[:, c, :], in_=wv_d[c * P:(c + 1) * P, :])
        for c in (0, 2, 1, 3):
            x_engs[c].dma_start(out=xT[:, c, 0:QLEN],
                                in_=xT_d[c * P:(c + 1) * P, 0:QLEN])
        for c in range(N_DC):
            nc.gpsimd.dma_start(out=wk_t[:, c, :], in_=wk_d[c * P:(c + 1) * P, :])
        for c in (0, 2, 1, 3):
            x_engs[c].dma_start(out=xT[:, c, QLEN:S],
                                in_=xT_d[c * P:(c + 1) * P, QLEN:S])

        # ---- projections ------------------------------------------------
        qT = big.tile([P, N_DC, QLEN], F16, tag="qT")
        for do in range(N_DC):
            for qh in range(QLEN // 512):
                pp = ps_a.tile([P, 512], F32, tag="a", name="pp")
                for c in range(N_DC):
                    nc.tensor.matmul(
                        pp[:], wq_t[:, c, do * P:(do + 1) * P],
                        xT[:, c, qh * 512:(qh + 1) * 512],
                        start=(c == 0), stop=(c == N_DC - 1))
                nc.vector.tensor_copy(qT[:, do, qh * 512:(qh + 1) * 512], pp[:])

        kT = big.tile([P, N_DC, S], F16, tag="kT")
        vtA = big.tile([P, N_KT, 257], F16, tag="vtA")
        vtB = big.tile([P, N_KT, 256], F16, tag="vtB")
        nc.vector.memset(vtA[:, :, 256:257], 1.0)

        for blk in range(N_KB):
            k0 = blk * 512
            for do in range(N_DC):
                pp = ps_a.tile([P, 512], F32, tag="a", name="pp")
                for c in range(N_DC):
                    nc.tensor.matmul(
                        pp[:], wk_t[:, c, do * P:(do + 1) * P],
                        xT[:, c, k0:k0 + 512],
                        start=(c == 0), stop=(c == N_DC - 1))
                nc.vector.tensor_copy(kT[:, do, k0:k0 + 512], pp[:])
            for j in range(4):
                kt = blk * 4 + j
                pp = ps_a.tile([P, 512], F32, tag="a", name="pp")
                for c in range(N_DC):
                    nc.tensor.matmul(
                        pp[:], xT[:, c, kt * P:(kt + 1) * P], wv_t[:, c, :],
                        start=(c == 0), stop=(c == N_DC - 1))
                nc.vector.tensor_copy(vtA[:, kt, 0:256], pp[:, 0:256])
                nc.scalar.copy(vtB[:, kt, :], pp[:, 256:512])

        # ---- attention --------------------------------------------------
        st_engs = (nc.sync, nc.gpsimd)
        for qb in range(N_QB):
            q0 = qb * QB
            cxA = ps_cx.tile([P, 257], F32, tag="cxA", name="cxA")
            cxB = ps_cx.tile([P, 257], F32, tag="cxB", name="cxB")
            cxP0 = ps_cx.tile([P, 256], F32, tag="cxP0", name="cxP0")
            cxP1 = ps_cx.tile([P, 256], F32, tag="cxP1", name="cxP1")
            probs = {}

            def emit_sc(kt):
                psc = ps_a.tile([P, QB], F32, tag="a", name="psc")
                for c in range(N_DC):
                    nc.tensor.matmul(
                        psc[:], kT[:, c, kt * P:(kt + 1) * P],
                        qT[:, c, q0:q0 + QB],
                        start=(c == 0), stop=(c == N_DC - 1))
                pr = pr_pool.tile([P, QB], F16, tag="pr", name="pr")
                nc.scalar.activation(pr[:], psc[:], AF.Exp)
                probs[kt] = pr

            def emit_pv(kt):
                pr = probs.pop(kt)
                first = kt == 0
                last = kt == N_KT - 1
                nc.tensor.matmul(cxA[:], pr[:, 0:P], vtA[:, kt, :],
                                 start=first, stop=last)
                nc.tensor.matmul(cxP0[:], pr[:, 0:P], vtB[:, kt, :],
                                 start=first, stop=last)
                nc.tensor.matmul(cxB[:], pr[:, P:QB], vtA[:, kt, :],
                                 start=first, stop=last)
                nc.tensor.matmul(cxP1[:], pr[:, P:QB], vtB[:, kt, :],
                                 start=first, stop=last)

            emit_sc(0)
            emit_sc(1)
            for kt in range(N_KT):
                if kt + 2 < N_KT:
                    emit_sc(kt + 2)
                emit_pv(kt)

            recipA = st_pool.tile([P, 1], F32, tag="recipA", name="recipA")
            nc.vector.reciprocal(recipA[:], cxA[:, 256:257])
            recipB = st_pool.tile([P, 1], F32, tag="recipB", name="recipB")
            nc.vector.reciprocal(recipB[:], cxB[:, 256:257])
            osb0 = st_pool.tile([P, D], F32, tag="osb0", name="osb0")
            osb1 = st_pool.tile([P, D], F32, tag="osb1", name="osb1")
            nc.vector.tensor_scalar_mul(osb0[:, 0:256], cxA[:, 0:256],
                                        recipA[:])
            nc.scalar.mul(osb1[:, 0:256], cxB[:, 0:256], recipB[:])
            nc.vector.tensor_scalar_mul(osb0[:, 256:512], cxP0[:], recipA[:])
            nc.scalar.mul(osb1[:, 256:512], cxP1[:], recipB[:])
            nc.sync.dma_start(out=out_d[q0:q0 + P, :], in_=osb0[:])
            nc.gpsimd.dma_start(out=out_d[q0 + P:q0 + 2 * P, :], in_=osb1[:])

    nc.compile()
    return nc


_CACHE = {}


def _get_nc(nreps=1):
    if nreps not in _CACHE:
        _CACHE[nreps] = _build(nreps)
    return _CACHE[nreps]


def _in_maps(x, W_query, W_key, W_value):
    x = np.asarray(x, dtype=np.float32)
    wq = np.ascontiguousarray(
        np.asarray(W_query, dtype=np.float32) * SCALE).astype(np.float16)
    wk = np.ascontiguousarray(np.asarray(W_key, dtype=np.float32)).astype(
        np.float16)
    wv = np.ascontiguousarray(np.asarray(W_value, dtype=np.float32)).astype(
        np.float16)
    maps = []
    for core in range(N_CORES):
        b = core // CORES_PER_B
        q0 = (core % CORES_PER_B) * QLEN
        xT = np.ascontiguousarray(
            np.roll(x[b], -q0, axis=0).T.astype(np.float16))
        maps.append({"xT": xT, "wq": wq, "wk": wk, "wv": wv})
    return maps


def kernel(x, W_query, W_key, W_value, _trace=False):
    import os
    if not _trace:
        # NTFF tracing is unavailable here; make sure an inherited
        # BASS_TRACE can't route execution down that path.
        os.environ.setdefault("BASS_NEVER_TRACE", "1")
    nc = _get_nc()
    maps = _in_maps(x, W_query, W_key, W_value)
    res = run_bass_kernel_spmd(nc, maps, list(range(N_CORES)), trace=_trace)
    out = np.empty((B, S, D), dtype=np.float32)
    for core in range(N_CORES):
        b = core // CORES_PER_B
        q0 = (core % CORES_PER_B) * QLEN
        out[b, q0:q0 + QLEN] = res.results[core]["out"]
    if _trace:
        return out, res
    return out


# revision 8
# speedup vs baseline: 1.5132x; 1.2554x over previous
"""Full-attention kernel (QKV projections + softmax(QK^T/sqrt(d))V) on 8
trn2 NeuronCores.

Problem: x [2,4096,512] f32, W_{q,k,v} [512,512] f32 -> context [2,4096,512]
f32 (the reference applies no causal mask and dropout=0).

Distribution (data parallel, no collectives): core c handles batch b = c // 4
and query block q0 = (c % 4) * 1024; the host rotates each core's copy of
x[b] so its query rows come first (attention is permutation-invariant over
keys) and ships both x and x^T in fp16.

Algebraic restructure (the big lever): per core, queries (1024) are 4x fewer
than keys (4096), so every weight application is folded onto the query side:
  * scores   s = x_q^T (Wq Wk^T / sqrt(d)) x_k = qt . x_k  with
    qt = M^T x_q, M = (Wq Wk^T) / sqrt(d) precomputed on the host in f64.
    -> NO key projection; x^T itself is the transposed-key matmul operand.
  * context  ctx = P^T X Wv = (P^T X) Wv: accumulate ct = P^T X against raw
    x, then apply Wv once per 128-query block. -> NO value projection.
PE work per core: qt 16.4k + scores 131k + P^T X 131.3k + ct transposes
4.1k + Wv apply 16.4k ~= 300k cycles (vs 483k direct / 410k with separate
K,V projections).

Layout/precision:
  * scores are computed TRANSPOSED, [k_tile=128, q=256]: lhsT = x^T chunks,
    rhs = qt chunks; exp(scores^T) on ACT feeds the P^T X matmul directly
    as stationary -- no probability transposes.
  * the softmax row-sum is folded into the P^T X matmul via a ones-column
    appended to raw x (257-col tiles): each query's sum(exp) accumulates in
    psum column 256, landing per-partition exactly where the 1/rowsum
    normalization needs it (cost: 1 extra psum column per matmul).
  * fp16 operands everywhere (1 cyc/row on PE), f32 psum accumulation,
    f32 output.
"""
import numpy as np
from contextlib import ExitStack

from concourse import bacc
import concourse.mybir as mybir
import concourse.tile as tile
from concourse.bass_utils import run_bass_kernel_spmd
from concourse.masks import make_identity

F32 = mybir.dt.float32
F32R = mybir.dt.float32r
F16 = mybir.dt.float16
AF = mybir.ActivationFunctionType

B, S, D = 2, 4096, 512
N_CORES = 8
CORES_PER_B = N_CORES // B
QLEN = S // CORES_PER_B             # 1024
P = 128
SCALE = 1.0 / float(np.sqrt(D))

N_DC = D // P                       # 4 contraction chunks
N_KT = S // P                       # 32 key tiles
QB = 256                            # query block (psum-bank friendly)
N_QB = QLEN // QB                   # 4


def _build(nreps=1):
    nc = bacc.Bacc(None)
    xT_d = nc.declare_dram_parameter("xT", [D, S], F16, isOutput=False)
    # raw x pre-swizzled on host to [P, N_KT, D]: xsw[p, kt, :] = x[kt*128+p]
    xsw_d = nc.declare_dram_parameter("xsw", [P, N_KT, D], F16, isOutput=False)
    m_d = nc.declare_dram_parameter("m", [D, D], F16, isOutput=False)
    wv_d = nc.declare_dram_parameter("wv", [D, D], F16, isOutput=False)
    out_d = nc.declare_dram_parameter("out", [QLEN, D], F32, isOutput=True)

    with tile.TileContext(nc) as tc, ExitStack() as ctx:
        const = ctx.enter_context(tc.tile_pool(name="const", bufs=1))
        big = ctx.enter_context(tc.tile_pool(name="big", bufs=1))
        pr_pool = ctx.enter_context(tc.tile_pool(name="pr", bufs=4))
        st_pool = ctx.enter_context(tc.tile_pool(name="st", bufs=2))
        ct_pool = ctx.enter_context(tc.tile_pool(name="ct", bufs=2))
        # psum budget (8 banks): "a" ring 3 + cx (2+1+1) + tr 1
        ps_a = ctx.enter_context(tc.tile_pool(name="ps_a", bufs=3, space="PSUM"))
        ps_cx = ctx.enter_context(tc.tile_pool(name="ps_cx", bufs=1, space="PSUM"))
        ps_tr = ctx.enter_context(tc.tile_pool(name="ps_tr", bufs=1, space="PSUM"))

        ident_f = const.tile([P, P], F32)
        make_identity(nc, ident_f[:])
        ident = const.tile([P, P], F16)
        nc.vector.tensor_copy(ident[:], ident_f[:])

        # ---- input DMAs -------------------------------------------------
        m_t = big.tile([P, N_DC, D], F16, tag="m")
        wv_t = big.tile([P, N_DC, D], F16, tag="wv")
        xT = big.tile([P, N_DC, S], F16, tag="xT")
        xA = big.tile([P, N_KT, 257], F16, tag="xA")
        xB = big.tile([P, N_KT, 256], F16, tag="xB")
        nc.vector.memset(xA[:, :, 256:257], 1.0)

        # queue plan: sync/scalar carry M then xT (query cols first);
        # gpsimd carries wv then the two big raw-x swizzled loads.
        x_engs = (nc.sync, nc.sync, nc.scalar, nc.scalar)
        nc.sync.dma_start(out=m_t[:, 0, :], in_=m_d[0:P, :])
        nc.sync.dma_start(out=m_t[:, 1, :], in_=m_d[P:2 * P, :])
        nc.scalar.dma_start(out=m_t[:, 2, :], in_=m_d[2 * P:3 * P, :])
        nc.scalar.dma_start(out=m_t[:, 3, :], in_=m_d[3 * P:4 * P, :])
        for c in (0, 2, 1, 3):
            x_engs[c].dma_start(out=xT[:, c, 0:512],
                                in_=xT_d[c * P:(c + 1) * P, 0:512])
        for c in range(N_DC):
            nc.gpsimd.dma_start(out=wv_t[:, c, :], in_=wv_d[c * P:(c + 1) * P, :])
        nc.gpsimd.dma_start(out=xA[:, :, 0:256], in_=xsw_d[:, :, 0:256])
        nc.gpsimd.dma_start(out=xB[:, :, :], in_=xsw_d[:, :, 256:512])
        for c in (0, 2, 1, 3):
            x_engs[c].dma_start(out=xT[:, c, 512:1024],
                                in_=xT_d[c * P:(c + 1) * P, 512:1024])
        for c in (0, 2, 1, 3):
            x_engs[c].dma_start(out=xT[:, c, 1024:S],
                                in_=xT_d[c * P:(c + 1) * P, 1024:S])

        # ---- query transform qt = M^T x_q -------------------------------
        qT = big.tile([P, N_DC, QLEN], F16, tag="qT")
        for qh in range(QLEN // 512):
            for do in range(N_DC):
                pp = ps_a.tile([P, 512], F32, tag="a", name="pp")
                for c in range(N_DC):
                    nc.tensor.matmul(
                        pp[:], m_t[:, c, do * P:(do + 1) * P],
                        xT[:, c, qh * 512:(qh + 1) * 512],
                        start=(c == 0), stop=(c == N_DC - 1))
                nc.vector.tensor_copy(qT[:, do, qh * 512:(qh + 1) * 512], pp[:])

        # ---- attention --------------------------------------------------
        for qb in range(N_QB):
            q0 = qb * QB
            cxA = ps_cx.tile([P, 257], F32, tag="cxA", name="cxA")
            cxB = ps_cx.tile([P, 257], F32, tag="cxB", name="cxB")
            cxP0 = ps_cx.tile([P, 256], F32, tag="cxP0", name="cxP0")
            cxP1 = ps_cx.tile([P, 256], F32, tag="cxP1", name="cxP1")
            probs = {}

            def emit_sc(kt):
                psc = ps_a.tile([P, QB], F32, tag="a", name="psc")
                for c in range(N_DC):
                    nc.tensor.matmul(
                        psc[:], xT[:, c, kt * P:(kt + 1) * P],
                        qT[:, c, q0:q0 + QB],
                        start=(c == 0), stop=(c == N_DC - 1))
                pr = pr_pool.tile([P, QB], F16, tag="pr", name="pr")
                nc.scalar.activation(pr[:], psc[:], AF.Exp)
                probs[kt] = pr

            def emit_pv(kt):
                pr = probs.pop(kt)
                first = kt == 0
                last = kt == N_KT - 1
                nc.tensor.matmul(cxA[:], pr[:, 0:P], xA[:, kt, :],
                                 start=first, stop=last)
                nc.tensor.matmul(cxP0[:], pr[:, 0:P], xB[:, kt, :],
                                 start=first, stop=last)
                nc.tensor.matmul(cxB[:], pr[:, P:QB], xA[:, kt, :],
                                 start=first, stop=last)
                nc.tensor.matmul(cxP1[:], pr[:, P:QB], xB[:, kt, :],
                                 start=first, stop=last)

            emit_sc(0)
            emit_sc(1)
            for kt in range(N_KT):
                if kt + 2 < N_KT:
                    emit_sc(kt + 2)
                emit_pv(kt)

            # ---- flush: recip, ct copies, transpose, Wv apply, store ----
            recipA = st_pool.tile([P, 1], F32, tag="recipA", name="recipA")
            nc.vector.reciprocal(recipA[:], cxA[:, 256:257])
            recipB = st_pool.tile([P, 1], F32, tag="recipB", name="recipB")
            nc.vector.reciprocal(recipB[:], cxB[:, 256:257])
            # ct[qsub] = [P, 512] fp16 unnormalized P^T X for 128 queries;
            # qsub0 flush runs on DVE, qsub1 on ACT, in parallel.
            ct0 = ct_pool.tile([P, D], F16, tag="ct0", name="ct0")
            ct1 = ct_pool.tile([P, D], F16, tag="ct1", name="ct1")
            nc.vector.tensor_copy(ct0[:, 0:256], cxA[:, 0:256])
            nc.scalar.copy(ct1[:, 0:256], cxB[:, 0:256])
            nc.vector.tensor_copy(ct0[:, 256:512], cxP0[:])
            nc.scalar.copy(ct1[:, 256:512], cxP1[:])
            for h, (ct, recip, st_eng) in enumerate(
                    ((ct0, recipA, nc.sync), (ct1, recipB, nc.gpsimd))):
                trp = ps_tr.tile([P, N_DC, P], F16, tag="tr", name="trp")
                for c in range(N_DC):
                    nc.tensor.matmul(trp[:, c, :], ct[:, c * P:(c + 1) * P],
                                     ident[:], is_transpose=True,
                                     start=True, stop=True)
                ctT = ct_pool.tile([P, N_DC, P], F16, tag=f"ctT{h}",
                                   name="ctT")
                if h == 0:
                    nc.vector.tensor_copy(ctT[:], trp[:])
                else:
                    nc.scalar.copy(ctT[:], trp[:])
                osb = st_pool.tile([P, D], F32, tag=f"osb{h}", name="osb")
                for half in range(2):
                    po = ps_a.tile([P, 256], F32, tag="a", name="po")
                    for c in range(N_DC):
                        nc.tensor.matmul(
                            po[:], ctT[:, c, :],
                            wv_t[:, c, half * 256:(half + 1) * 256],
                            start=(c == 0), stop=(c == N_DC - 1))
                    if h == 0:
                        nc.vector.tensor_scalar_mul(
                            osb[:, half * 256:(half + 1) * 256], po[:],
                            recip[:])
                    else:
                        nc.scalar.mul(
                            osb[:, half * 256:(half + 1) * 256], po[:],
                            recip[:])
                qrow = q0 + h * P
                st_eng.dma_start(out=out_d[qrow:qrow + P, :], in_=osb[:])

    nc.compile()
    return nc


_CACHE = {}


def _get_nc(nreps=1):
    if nreps not in _CACHE:
        _CACHE[nreps] = _build(nreps)
    return _CACHE[nreps]


def _in_maps(x, W_query, W_key, W_value):
    x = np.asarray(x, dtype=np.float32)
    wq64 = np.asarray(W_query, dtype=np.float64)
    wk64 = np.asarray(W_key, dtype=np.float64)
    m = np.ascontiguousarray((wq64 @ wk64.T) * SCALE).astype(np.float16)
    wv = np.ascontiguousarray(np.asarray(W_value, dtype=np.float32)).astype(
        np.float16)
    maps = []
    for core in range(N_CORES):
        b = core // CORES_PER_B
        q0 = (core % CORES_PER_B) * QLEN
        xr = np.roll(x[b], -q0, axis=0)
        xT = np.ascontiguousarray(xr.T.astype(np.float16))
        xsw = np.ascontiguousarray(
            xr.astype(np.float16).reshape(N_KT, P, D).transpose(1, 0, 2))
        maps.append({"xT": xT, "xsw": xsw, "m": m, "wv": wv})
    return maps


def kernel(x, W_query, W_key, W_value, _trace=False):
    import os
    if not _trace:
        # NTFF tracing is unavailable here; make sure an inherited
        # BASS_TRACE can't route execution down that path.
        os.environ.setdefault("BASS_NEVER_TRACE", "1")
    nc = _get_nc()
    maps = _in_maps(x, W_query, W_key, W_value)
    res = run_bass_kernel_spmd(nc, maps, list(range(N_CORES)), trace=_trace)
    out = np.empty((B, S, D), dtype=np.float32)
    for core in range(N_CORES):
        b = core // CORES_PER_B
        q0 = (core % CORES_PER_B) * QLEN
        out[b, q0:q0 + QLEN] = res.results[core]["out"]
    if _trace:
        return out, res
    return out


# revision 9
# speedup vs baseline: 1.5132x; 1.0000x over previous
"""Full-attention kernel (QKV projections + softmax(QK^T/sqrt(d))V) on 8
trn2 NeuronCores.

Problem: x [2,4096,512] f32, W_{q,k,v} [512,512] f32 -> context [2,4096,512]
f32 (the reference applies no causal mask and dropout=0).

Distribution (data parallel, no collectives): core c handles batch b = c // 4
and query block q0 = (c % 4) * 1024; the host rotates each core's copy of
x[b] so its query rows come first (attention is permutation-invariant over
keys) and ships both x and x^T in fp16.

Algebraic restructure (the big lever): per core, queries (1024) are 4x fewer
than keys (4096), so every weight application is folded onto the query side:
  * scores   s = x_q^T (Wq Wk^T / sqrt(d)) x_k = qt . x_k  with
    qt = M^T x_q, M = (Wq Wk^T) / sqrt(d) precomputed on the host in f64.
    -> NO key projection; x^T itself is the transposed-key matmul operand.
  * context  ctx = P^T X Wv = (P^T X) Wv: accumulate ct = P^T X against raw
    x, then apply Wv once per 128-query block. -> NO value projection.
PE work per core: qt 16.4k + scores 131k + P^T X 131.3k + ct transposes
4.1k + Wv apply 16.4k ~= 300k cycles (vs 483k direct / 410k with separate
K,V projections).

Layout/precision:
  * scores are computed TRANSPOSED, [k_tile=128, q=256]: lhsT = x^T chunks,
    rhs = qt chunks; exp(scores^T) on ACT feeds the P^T X matmul directly
    as stationary -- no probability transposes.
  * the softmax row-sum is folded into the P^T X matmul via a ones-column
    appended to raw x (257-col tiles): each query's sum(exp) accumulates in
    psum column 256, landing per-partition exactly where the 1/rowsum
    normalization needs it (cost: 1 extra psum column per matmul).
  * fp16 operands everywhere (1 cyc/row on PE), f32 psum accumulation,
    f32 output.
"""
import numpy as np
from contextlib import ExitStack

from concourse import bacc
import concourse.mybir as mybir
import concourse.tile as tile
from concourse.bass_utils import run_bass_kernel_spmd
from concourse.masks import make_identity

F32 = mybir.dt.float32
F32R = mybir.dt.float32r
F16 = mybir.dt.float16
AF = mybir.ActivationFunctionType

B, S, D = 2, 4096, 512
N_CORES = 8
CORES_PER_B = N_CORES // B
QLEN = S // CORES_PER_B             # 1024
P = 128
SCALE = 1.0 / float(np.sqrt(D))

N_DC = D // P                       # 4 contraction chunks
N_KT = S // P                       # 32 key tiles
QB = 256                            # query block (psum-bank friendly)
N_QB = QLEN // QB                   # 4


def _build(nreps=1):
    nc = bacc.Bacc(None)
    xT_d = nc.declare_dram_parameter("xT", [D, S], F16, isOutput=False)
    # raw x pre-swizzled on host to [P, N_KT, D]: xsw[p, kt, :] = x[kt*128+p]
    xsw_d = nc.declare_dram_parameter("xsw", [P, N_KT, D], F16, isOutput=False)
    m_d = nc.declare_dram_parameter("m", [D, D], F16, isOutput=False)
    wv_d = nc.declare_dram_parameter("wv", [D, D], F16, isOutput=False)
    out_d = nc.declare_dram_parameter("out", [QLEN, D], F32, isOutput=True)

    with tile.TileContext(nc) as tc, ExitStack() as ctx:
        const = ctx.enter_context(tc.tile_pool(name="const", bufs=1))
        big = ctx.enter_context(tc.tile_pool(name="big", bufs=1))
        pr_pool = ctx.enter_context(tc.tile_pool(name="pr", bufs=4))
        st_pool = ctx.enter_context(tc.tile_pool(name="st", bufs=2))
        ct_pool = ctx.enter_context(tc.tile_pool(name="ct", bufs=2))
        # psum budget (8 banks): "a" ring 3 + cx (2+1+1) + tr 1
        ps_a = ctx.enter_context(tc.tile_pool(name="ps_a", bufs=3, space="PSUM"))
        ps_cx = ctx.enter_context(tc.tile_pool(name="ps_cx", bufs=1, space="PSUM"))
        ps_tr = ctx.enter_context(tc.tile_pool(name="ps_tr", bufs=1, space="PSUM"))

        ident_f = const.tile([P, P], F32)
        make_identity(nc, ident_f[:])
        ident = const.tile([P, P], F16)
        nc.vector.tensor_copy(ident[:], ident_f[:])

        # ---- input DMAs -------------------------------------------------
        m_t = big.tile([P, N_DC, D], F16, tag="m")
        wv_t = big.tile([P, N_DC, D], F16, tag="wv")
        xT = big.tile([P, N_DC, S], F16, tag="xT")
        xA = big.tile([P, N_KT, 257], F16, tag="xA")
        xB = big.tile([P, N_KT, 256], F16, tag="xB")
        nc.vector.memset(xA[:, :, 256:257], 1.0)

        # queue plan: sync/scalar carry M then xT (query cols first);
        # gpsimd carries wv then the two big raw-x swizzled loads.
        nc.sync.dma_start(out=m_t[:, 0, :], in_=m_d[0:P, :])
        nc.sync.dma_start(out=m_t[:, 1, :], in_=m_d[P:2 * P, :])
        nc.scalar.dma_start(out=m_t[:, 2, :], in_=m_d[2 * P:3 * P, :])
        nc.scalar.dma_start(out=m_t[:, 3, :], in_=m_d[3 * P:4 * P, :])
        for c in range(N_DC):
            nc.sync.dma_start(out=xT[:, c, 0:512],
                              in_=xT_d[c * P:(c + 1) * P, 0:512])
        for c in range(N_DC):
            nc.gpsimd.dma_start(out=wv_t[:, c, :], in_=wv_d[c * P:(c + 1) * P, :])
        nc.gpsimd.dma_start(out=xA[:, :, 0:256], in_=xsw_d[:, :, 0:256])
        nc.gpsimd.dma_start(out=xB[:, :, :], in_=xsw_d[:, :, 256:512])
        for c in range(N_DC):
            nc.sync.dma_start(out=xT[:, c, 512:1024],
                              in_=xT_d[c * P:(c + 1) * P, 512:1024])
        for c, eng in ((0, nc.sync), (1, nc.sync), (2, nc.gpsimd),
                       (3, nc.gpsimd)):
            eng.dma_start(out=xT[:, c, 1024:S],
                          in_=xT_d[c * P:(c + 1) * P, 1024:S])

        # ---- query transform qt = M^T x_q -------------------------------
        qT = big.tile([P, N_DC, QLEN], F16, tag="qT")
        for qh in range(QLEN // 512):
            for do in range(N_DC):
                pp = ps_a.tile([P, 512], F32, tag="a", name="pp")
                for c in range(N_DC):
                    nc.tensor.matmul(
                        pp[:], m_t[:, c, do * P:(do + 1) * P],
                        xT[:, c, qh * 512:(qh + 1) * 512],
                        start=(c == 0), stop=(c == N_DC - 1))
                nc.vector.tensor_copy(qT[:, do, qh * 512:(qh + 1) * 512], pp[:])

        # ---- attention --------------------------------------------------
        for qb in range(N_QB):
            q0 = qb * QB
            cxA = ps_cx.tile([P, 257], F32, tag="cxA", name="cxA")
            cxB = ps_cx.tile([P, 257], F32, tag="cxB", name="cxB")
            cxP0 = ps_cx.tile([P, 256], F32, tag="cxP0", name="cxP0")
            cxP1 = ps_cx.tile([P, 256], F32, tag="cxP1", name="cxP1")
            probs = {}

            def emit_sc(kt):
                psc = ps_a.tile([P, QB], F32, tag="a", name="psc")
                for c in range(N_DC):
                    nc.tensor.matmul(
                        psc[:], xT[:, c, kt * P:(kt + 1) * P],
                        qT[:, c, q0:q0 + QB],
                        start=(c == 0), stop=(c == N_DC - 1))
                pr = pr_pool.tile([P, QB], F16, tag="pr", name="pr")
                nc.scalar.activation(pr[:], psc[:], AF.Exp)
                probs[kt] = pr

            def emit_pv(kt):
                pr = probs.pop(kt)
                first = kt == 0
                last = kt == N_KT - 1
                nc.tensor.matmul(cxA[:], pr[:, 0:P], xA[:, kt, :],
                                 start=first, stop=last)
                nc.tensor.matmul(cxP0[:], pr[:, 0:P], xB[:, kt, :],
                                 start=first, stop=last)
                nc.tensor.matmul(cxB[:], pr[:, P:QB], xA[:, kt, :],
                                 start=first, stop=last)
                nc.tensor.matmul(cxP1[:], pr[:, P:QB], xB[:, kt, :],
                                 start=first, stop=last)

            emit_sc(0)
            emit_sc(1)
            for kt in range(N_KT):
                if kt + 2 < N_KT:
                    emit_sc(kt + 2)
                emit_pv(kt)

            # ---- flush: recip, ct copies, transpose, Wv apply, store ----
            recipA = st_pool.tile([P, 1], F32, tag="recipA", name="recipA")
            nc.vector.reciprocal(recipA[:], cxA[:, 256:257])
            recipB = st_pool.tile([P, 1], F32, tag="recipB", name="recipB")
            nc.vector.reciprocal(recipB[:], cxB[:, 256:257])
            # ct[qsub] = [P, 512] fp16 unnormalized P^T X for 128 queries;
            # qsub0 flush runs on DVE, qsub1 on ACT, in parallel.
            ct0 = ct_pool.tile([P, D], F16, tag="ct0", name="ct0")
            ct1 = ct_pool.tile([P, D], F16, tag="ct1", name="ct1")
            nc.vector.tensor_copy(ct0[:, 0:256], cxA[:, 0:256])
            nc.scalar.copy(ct1[:, 0:256], cxB[:, 0:256])
            nc.vector.tensor_copy(ct0[:, 256:512], cxP0[:])
            nc.scalar.copy(ct1[:, 256:512], cxP1[:])
            for h, (ct, recip, st_eng) in enumerate(
                    ((ct0, recipA, nc.sync), (ct1, recipB, nc.gpsimd))):
                trp = ps_tr.tile([P, N_DC, P], F16, tag="tr", name="trp")
                for c in range(N_DC):
                    nc.tensor.matmul(trp[:, c, :], ct[:, c * P:(c + 1) * P],
                                     ident[:], is_transpose=True,
                                     start=True, stop=True)
                ctT = ct_pool.tile([P, N_DC, P], F16, tag=f"ctT{h}",
                                   name="ctT")
                if h == 0:
                    nc.vector.tensor_copy(ctT[:], trp[:])
                else:
                    nc.scalar.copy(ctT[:], trp[:])
                osb = st_pool.tile([P, D], F32, tag=f"osb{h}", name="osb")
                for half in range(2):
                    po = ps_a.tile([P, 256], F32, tag="a", name="po")
                    for c in range(N_DC):
                        nc.tensor.matmul(
                            po[:], ctT[:, c, :],
                            wv_t[:, c, half * 256:(half + 1) * 256],
                            start=(c == 0), stop=(c == N_DC - 1))
                    if h == 0:
                        nc.vector.tensor_scalar_mul(
                            osb[:, half * 256:(half + 1) * 256], po[:],
                            recip[:])
                    else:
                        nc.scalar.mul(
                            osb[:, half * 256:(half + 1) * 256], po[:],
                            recip[:])
                qrow = q0 + h * P
                for half in range(2):
                    cols = slice(half * 256, (half + 1) * 256)
                    st_eng.dma_start(out=out_d[qrow:qrow + P, cols],
                                     in_=osb[:, cols])

    nc.compile()
    return nc


_CACHE = {}


def _get_nc(nreps=1):
    if nreps not in _CACHE:
        _CACHE[nreps] = _build(nreps)
    return _CACHE[nreps]


def _in_maps(x, W_query, W_key, W_value):
    x = np.asarray(x, dtype=np.float32)
    wq64 = np.asarray(W_query, dtype=np.float64)
    wk64 = np.asarray(W_key, dtype=np.float64)
    m = np.ascontiguousarray((wq64 @ wk64.T) * SCALE).astype(np.float16)
    wv = np.ascontiguousarray(np.asarray(W_value, dtype=np.float32)).astype(
        np.float16)
    maps = []
    for core in range(N_CORES):
        b = core // CORES_PER_B
        q0 = (core % CORES_PER_B) * QLEN
        xr = np.roll(x[b], -q0, axis=0)
        xT = np.ascontiguousarray(xr.T.astype(np.float16))
        xsw = np.ascontiguousarray(
            xr.astype(np.float16).reshape(N_KT, P, D).transpose(1, 0, 2))
        maps.append({"xT": xT, "xsw": xsw, "m": m, "wv": wv})
    return maps


def kernel(x, W_query, W_key, W_value, _trace=False):
    import os
    if not _trace:
        # NTFF tracing is unavailable here; make sure an inherited
        # BASS_TRACE can't route execution down that path.
        os.environ.setdefault("BASS_NEVER_TRACE", "1")
    nc = _get_nc()
    maps = _in_maps(x, W_query, W_key, W_value)
    res = run_bass_kernel_spmd(nc, maps, list(range(N_CORES)), trace=_trace)
    out = np.empty((B, S, D), dtype=np.float32)
    for core in range(N_CORES):
        b = core // CORES_PER_B
        q0 = (core % CORES_PER_B) * QLEN
        out[b, q0:q0 + QLEN] = res.results[core]["out"]
    if _trace:
        return out, res
    return out


# revision 10
# speedup vs baseline: 1.5282x; 1.0099x over previous
"""Full-attention kernel (QKV projections + softmax(QK^T/sqrt(d))V) on 8
trn2 NeuronCores.

Problem: x [2,4096,512] f32, W_{q,k,v} [512,512] f32 -> context [2,4096,512]
f32 (the reference applies no causal mask and dropout=0).

Distribution (data parallel, no collectives): core c handles batch b = c // 4
and query block q0 = (c % 4) * 1024; the host rotates each core's copy of
x[b] so its query rows come first (attention is permutation-invariant over
keys) and ships both x and x^T in fp16.

Algebraic restructure (the big lever): per core, queries (1024) are 4x fewer
than keys (4096), so every weight application is folded onto the query side:
  * scores   s = x_q^T (Wq Wk^T / sqrt(d)) x_k = qt . x_k  with
    qt = M^T x_q, M = (Wq Wk^T) / sqrt(d) precomputed on the host in f64.
    -> NO key projection; x^T itself is the transposed-key matmul operand.
  * context  ctx = P^T X Wv = (P^T X) Wv: accumulate ct = P^T X against raw
    x, then apply Wv once per 128-query block. -> NO value projection.
PE work per core: qt 16.4k + scores 131k + P^T X 131.3k + ct transposes
4.1k + Wv apply 16.4k ~= 300k cycles (vs 483k direct / 410k with separate
K,V projections).

Layout/precision:
  * scores are computed TRANSPOSED, [k_tile=128, q=256]: lhsT = x^T chunks,
    rhs = qt chunks; exp(scores^T) on ACT feeds the P^T X matmul directly
    as stationary -- no probability transposes.
  * the softmax row-sum is folded into the P^T X matmul via a ones-column
    appended to raw x (257-col tiles): each query's sum(exp) accumulates in
    psum column 256, landing per-partition exactly where the 1/rowsum
    normalization needs it (cost: 1 extra psum column per matmul).
  * fp16 operands everywhere (1 cyc/row on PE), f32 psum accumulation,
    f32 output.
"""
import numpy as np
from contextlib import ExitStack

from concourse import bacc
import concourse.mybir as mybir
import concourse.tile as tile
from concourse.bass_utils import run_bass_kernel_spmd
from concourse.masks import make_identity

F32 = mybir.dt.float32
F32R = mybir.dt.float32r
F16 = mybir.dt.float16
AF = mybir.ActivationFunctionType

B, S, D = 2, 4096, 512
N_CORES = 8
CORES_PER_B = N_CORES // B
QLEN = S // CORES_PER_B             # 1024
P = 128
SCALE = 1.0 / float(np.sqrt(D))

N_DC = D // P                       # 4 contraction chunks
N_KT = S // P                       # 32 key tiles
QB = 256                            # query block (psum-bank friendly)
N_QB = QLEN // QB                   # 4


def _build(nreps=1):
    nc = bacc.Bacc(None)
    xT_d = nc.declare_dram_parameter("xT", [D, S], F16, isOutput=False)
    # raw x pre-swizzled on host to [P, N_KT, D]: xsw[p, kt, :] = x[kt*128+p]
    xsw_d = nc.declare_dram_parameter("xsw", [P, N_KT, D], F16, isOutput=False)
    m_d = nc.declare_dram_parameter("m", [D, D], F16, isOutput=False)
    wv_d = nc.declare_dram_parameter("wv", [D, D], F16, isOutput=False)
    out_d = nc.declare_dram_parameter("out", [QLEN, D], F32, isOutput=True)

    with tile.TileContext(nc) as tc, ExitStack() as ctx:
        const = ctx.enter_context(tc.tile_pool(name="const", bufs=1))
        big = ctx.enter_context(tc.tile_pool(name="big", bufs=1))
        pr_pool = ctx.enter_context(tc.tile_pool(name="pr", bufs=4))
        st_pool = ctx.enter_context(tc.tile_pool(name="st", bufs=2))
        ct_pool = ctx.enter_context(tc.tile_pool(name="ct", bufs=2))
        # psum budget (8 banks): "a" ring 3 + cx (2+1+1) + tr 1
        ps_a = ctx.enter_context(tc.tile_pool(name="ps_a", bufs=3, space="PSUM"))
        ps_cx = ctx.enter_context(tc.tile_pool(name="ps_cx", bufs=1, space="PSUM"))
        ps_tr = ctx.enter_context(tc.tile_pool(name="ps_tr", bufs=1, space="PSUM"))

        ident_f = const.tile([P, P], F32)
        make_identity(nc, ident_f[:])
        ident = const.tile([P, P], F16)
        nc.vector.tensor_copy(ident[:], ident_f[:])

        # ---- input DMAs -------------------------------------------------
        m_t = big.tile([P, N_DC, D], F16, tag="m")
        wv_t = big.tile([P, N_DC, D], F16, tag="wv")
        xT = big.tile([P, N_DC, S], F16, tag="xT")
        xA = big.tile([P, N_KT, 257], F16, tag="xA")
        xB = big.tile([P, N_KT, 256], F16, tag="xB")
        nc.vector.memset(xA[:, :, 256:257], 1.0)

        # queue plan: sync/scalar carry M then xT (query cols first);
        # gpsimd carries wv then the two big raw-x swizzled loads.
        nc.sync.dma_start(out=m_t[:, 0, :], in_=m_d[0:P, :])
        nc.sync.dma_start(out=m_t[:, 1, :], in_=m_d[P:2 * P, :])
        nc.scalar.dma_start(out=m_t[:, 2, :], in_=m_d[2 * P:3 * P, :])
        nc.scalar.dma_start(out=m_t[:, 3, :], in_=m_d[3 * P:4 * P, :])
        for c in range(N_DC):
            nc.sync.dma_start(out=xT[:, c, 0:512],
                              in_=xT_d[c * P:(c + 1) * P, 0:512])
        for c in range(N_DC):
            nc.gpsimd.dma_start(out=wv_t[:, c, :], in_=wv_d[c * P:(c + 1) * P, :])
        # raw x in 8-keytile pieces so early key tiles land before PV starts
        KTC = 8
        for g in range(N_KT // KTC):
            ks = slice(g * KTC, (g + 1) * KTC)
            nc.gpsimd.dma_start(out=xA[:, ks, 0:256], in_=xsw_d[:, ks, 0:256])
            nc.gpsimd.dma_start(out=xB[:, ks, :], in_=xsw_d[:, ks, 256:512])
        for c in range(N_DC):
            nc.sync.dma_start(out=xT[:, c, 512:1024],
                              in_=xT_d[c * P:(c + 1) * P, 512:1024])
        for c, eng in ((0, nc.sync), (1, nc.sync), (2, nc.scalar),
                       (3, nc.scalar)):
            eng.dma_start(out=xT[:, c, 1024:S],
                          in_=xT_d[c * P:(c + 1) * P, 1024:S])

        # ---- query transform qt = M^T x_q -------------------------------
        qT = big.tile([P, N_DC, QLEN], F16, tag="qT")
        for qh in range(QLEN // 512):
            for do in range(N_DC):
                pp = ps_a.tile([P, 512], F32, tag="a", name="pp")
                for c in range(N_DC):
                    nc.tensor.matmul(
                        pp[:], m_t[:, c, do * P:(do + 1) * P],
                        xT[:, c, qh * 512:(qh + 1) * 512],
                        start=(c == 0), stop=(c == N_DC - 1))
                nc.vector.tensor_copy(qT[:, do, qh * 512:(qh + 1) * 512], pp[:])

        # ---- attention --------------------------------------------------
        for qb in range(N_QB):
            q0 = qb * QB
            cxA = ps_cx.tile([P, 257], F32, tag="cxA", name="cxA")
            cxB = ps_cx.tile([P, 257], F32, tag="cxB", name="cxB")
            cxP0 = ps_cx.tile([P, 256], F32, tag="cxP0", name="cxP0")
            cxP1 = ps_cx.tile([P, 256], F32, tag="cxP1", name="cxP1")
            probs = {}

            def emit_sc(kt):
                psc = ps_a.tile([P, QB], F32, tag="a", name="psc")
                for c in range(N_DC):
                    nc.tensor.matmul(
                        psc[:], xT[:, c, kt * P:(kt + 1) * P],
                        qT[:, c, q0:q0 + QB],
                        start=(c == 0), stop=(c == N_DC - 1))
                pr = pr_pool.tile([P, QB], F16, tag="pr", name="pr")
                nc.scalar.activation(pr[:], psc[:], AF.Exp)
                probs[kt] = pr

            def emit_pv(kt):
                pr = probs.pop(kt)
                first = kt == 0
                last = kt == N_KT - 1
                nc.tensor.matmul(cxA[:], pr[:, 0:P], xA[:, kt, :],
                                 start=first, stop=last)
                nc.tensor.matmul(cxP0[:], pr[:, 0:P], xB[:, kt, :],
                                 start=first, stop=last)
                nc.tensor.matmul(cxB[:], pr[:, P:QB], xA[:, kt, :],
                                 start=first, stop=last)
                nc.tensor.matmul(cxP1[:], pr[:, P:QB], xB[:, kt, :],
                                 start=first, stop=last)

            emit_sc(0)
            emit_sc(1)
            for kt in range(N_KT):
                if kt + 2 < N_KT:
                    emit_sc(kt + 2)
                emit_pv(kt)

            # ---- flush: recip, ct copies, transpose, Wv apply, store ----
            recipA = st_pool.tile([P, 1], F32, tag="recipA", name="recipA")
            nc.vector.reciprocal(recipA[:], cxA[:, 256:257])
            recipB = st_pool.tile([P, 1], F32, tag="recipB", name="recipB")
            nc.vector.reciprocal(recipB[:], cxB[:, 256:257])
            # ct[qsub] = [P, 512] fp16 unnormalized P^T X for 128 queries;
            # qsub0 flush runs on DVE, qsub1 on ACT, in parallel.
            ct0 = ct_pool.tile([P, D], F16, tag="ct0", name="ct0")
            ct1 = ct_pool.tile([P, D], F16, tag="ct1", name="ct1")
            nc.vector.tensor_copy(ct0[:, 0:256], cxA[:, 0:256])
            nc.scalar.copy(ct1[:, 0:256], cxB[:, 0:256])
            nc.vector.tensor_copy(ct0[:, 256:512], cxP0[:])
            nc.scalar.copy(ct1[:, 256:512], cxP1[:])
            for h, (ct, recip, st_eng) in enumerate(
                    ((ct0, recipA, nc.sync), (ct1, recipB, nc.gpsimd))):
                trp = ps_tr.tile([P, N_DC, P], F16, tag="tr", name="trp")
                for c in range(N_DC):
                    nc.tensor.matmul(trp[:, c, :], ct[:, c * P:(c + 1) * P],
                                     ident[:], is_transpose=True,
                                     start=True, stop=True)
                ctT = ct_pool.tile([P, N_DC, P], F16, tag=f"ctT{h}",
                                   name="ctT")
                if h == 0:
                    nc.vector.tensor_copy(ctT[:], trp[:])
                else:
                    nc.scalar.copy(ctT[:], trp[:])
                osb = st_pool.tile([P, D], F32, tag=f"osb{h}", name="osb")
                for half in range(2):
                    po = ps_a.tile([P, 256], F32, tag="a", name="po")
                    for c in range(N_DC):
                        nc.tensor.matmul(
                            po[:], ctT[:, c, :],
                            wv_t[:, c, half * 256:(half + 1) * 256],
                            start=(c == 0), stop=(c == N_DC - 1))
                    if h == 0:
                        nc.vector.tensor_scalar_mul(
                            osb[:, half * 256:(half + 1) * 256], po[:],
                            recip[:])
                    else:
                        nc.scalar.mul(
                            osb[:, half * 256:(half + 1) * 256], po[:],
                            recip[:])
                qrow = q0 + h * P
                for half in range(2):
                    cols = slice(half * 256, (half + 1) * 256)
                    st_eng.dma_start(out=out_d[qrow:qrow + P, cols],
                                     in_=osb[:, cols])

    nc.compile()
    return nc


_CACHE = {}


def _get_nc(nreps=1):
    if nreps not in _CACHE:
        _CACHE[nreps] = _build(nreps)
    return _CACHE[nreps]


def _in_maps(x, W_query, W_key, W_value):
    x = np.asarray(x, dtype=np.float32)
    wq64 = np.asarray(W_query, dtype=np.float64)
    wk64 = np.asarray(W_key, dtype=np.float64)
    m = np.ascontiguousarray((wq64 @ wk64.T) * SCALE).astype(np.float16)
    wv = np.ascontiguousarray(np.asarray(W_value, dtype=np.float32)).astype(
        np.float16)
    maps = []
    for core in range(N_CORES):
        b = core // CORES_PER_B
        q0 = (core % CORES_PER_B) * QLEN
        xr = np.roll(x[b], -q0, axis=0)
        xT = np.ascontiguousarray(xr.T.astype(np.float16))
        xsw = np.ascontiguousarray(
            xr.astype(np.float16).reshape(N_KT, P, D).transpose(1, 0, 2))
        maps.append({"xT": xT, "xsw": xsw, "m": m, "wv": wv})
    return maps


def kernel(x, W_query, W_key, W_value, _trace=False):
    import os
    if not _trace:
        # NTFF tracing is unavailable here; make sure an inherited
        # BASS_TRACE can't route execution down that path.
        os.environ.setdefault("BASS_NEVER_TRACE", "1")
    nc = _get_nc()
    maps = _in_maps(x, W_query, W_key, W_value)
    res = run_bass_kernel_spmd(nc, maps, list(range(N_CORES)), trace=_trace)
    out = np.empty((B, S, D), dtype=np.float32)
    for core in range(N_CORES):
        b = core // CORES_PER_B
        q0 = (core % CORES_PER_B) * QLEN
        out[b, q0:q0 + QLEN] = res.results[core]["out"]
    if _trace:
        return out, res
    return out


# revision 11
# speedup vs baseline: 1.5489x; 1.0136x over previous
"""Full-attention kernel (QKV projections + softmax(QK^T/sqrt(d))V) on 8
trn2 NeuronCores.

Problem: x [2,4096,512] f32, W_{q,k,v} [512,512] f32 -> context [2,4096,512]
f32 (the reference applies no causal mask and dropout=0).

Distribution (data parallel, no collectives): core c handles batch b = c // 4
and query block q0 = (c % 4) * 1024; the host rotates each core's copy of
x[b] so its query rows come first (attention is permutation-invariant over
keys) and ships both x and x^T in fp16.

Algebraic restructure (the big lever): per core, queries (1024) are 4x fewer
than keys (4096), so every weight application is folded onto the query side:
  * scores   s = x_q^T (Wq Wk^T / sqrt(d)) x_k = qt . x_k  with
    qt = M^T x_q, M = (Wq Wk^T) / sqrt(d) precomputed on the host in f64.
    -> NO key projection; x^T itself is the transposed-key matmul operand.
  * context  ctx = P^T X Wv = (P^T X) Wv: accumulate ct = P^T X against raw
    x, then apply Wv once per 128-query block. -> NO value projection.
PE work per core: qt 16.4k + scores 131k + P^T X 131.3k + ct transposes
4.1k + Wv apply 16.4k ~= 300k cycles (vs 483k direct / 410k with separate
K,V projections).

Layout/precision:
  * scores are computed TRANSPOSED, [k_tile=128, q=256]: lhsT = x^T chunks,
    rhs = qt chunks; exp(scores^T) on ACT feeds the P^T X matmul directly
    as stationary -- no probability transposes.
  * the softmax row-sum is folded into the P^T X matmul via a ones-column
    appended to raw x (257-col tiles): each query's sum(exp) accumulates in
    psum column 256, landing per-partition exactly where the 1/rowsum
    normalization needs it (cost: 1 extra psum column per matmul).
  * fp16 operands everywhere (1 cyc/row on PE), f32 psum accumulation,
    f32 output.
"""
import numpy as np
from contextlib import ExitStack

from concourse import bacc
import concourse.mybir as mybir
import concourse.tile as tile
from concourse.bass_utils import run_bass_kernel_spmd
from concourse.masks import make_identity

F32 = mybir.dt.float32
F32R = mybir.dt.float32r
F16 = mybir.dt.float16
AF = mybir.ActivationFunctionType

B, S, D = 2, 4096, 512
N_CORES = 8
CORES_PER_B = N_CORES // B
QLEN = S // CORES_PER_B             # 1024
P = 128
SCALE = 1.0 / float(np.sqrt(D))

N_DC = D // P                       # 4 contraction chunks
N_KT = S // P                       # 32 key tiles
QB = 256                            # query block (psum-bank friendly)
N_QB = QLEN // QB                   # 4


def _build(nreps=1):
    nc = bacc.Bacc(None)
    xT_d = nc.declare_dram_parameter("xT", [D, S], F16, isOutput=False)
    # raw x pre-swizzled on host to [P, N_KT, D]: xsw[p, kt, :] = x[kt*128+p]
    xsw_d = nc.declare_dram_parameter("xsw", [P, N_KT, D], F16, isOutput=False)
    m_d = nc.declare_dram_parameter("m", [D, D], F16, isOutput=False)
    wv_d = nc.declare_dram_parameter("wv", [D, D], F16, isOutput=False)
    out_d = nc.declare_dram_parameter("out", [QLEN, D], F32, isOutput=True)

    with tile.TileContext(nc) as tc, ExitStack() as ctx:
        const = ctx.enter_context(tc.tile_pool(name="const", bufs=1))
        big = ctx.enter_context(tc.tile_pool(name="big", bufs=1))
        pr_pool = ctx.enter_context(tc.tile_pool(name="pr", bufs=4))
        st_pool = ctx.enter_context(tc.tile_pool(name="st", bufs=2))
        ct_pool = ctx.enter_context(tc.tile_pool(name="ct", bufs=2))
        # psum budget (8 banks): "a" ring 3 + cx (2+1+1) + tr 1
        ps_a = ctx.enter_context(tc.tile_pool(name="ps_a", bufs=3, space="PSUM"))
        ps_cx = ctx.enter_context(tc.tile_pool(name="ps_cx", bufs=1, space="PSUM"))
        ps_tr = ctx.enter_context(tc.tile_pool(name="ps_tr", bufs=1, space="PSUM"))

        ident_f = const.tile([P, P], F32)
        make_identity(nc, ident_f[:])
        ident = const.tile([P, P], F16)
        nc.vector.tensor_copy(ident[:], ident_f[:])

        # ---- input DMAs -------------------------------------------------
        m_t = big.tile([P, N_DC, D], F16, tag="m")
        wv_t = big.tile([P, N_DC, D], F16, tag="wv")
        xT = big.tile([P, N_DC, S], F16, tag="xT")
        xA = big.tile([P, N_KT, 257], F16, tag="xA")
        xB = big.tile([P, N_KT, 256], F16, tag="xB")
        nc.vector.memset(xA[:, :, 256:257], 1.0)

        # queue plan: sync/scalar carry M then xT (query cols first);
        # gpsimd carries wv then the two big raw-x swizzled loads.
        nc.sync.dma_start(out=m_t[:, 0, :], in_=m_d[0:P, :])
        nc.sync.dma_start(out=m_t[:, 1, :], in_=m_d[P:2 * P, :])
        nc.scalar.dma_start(out=m_t[:, 2, :], in_=m_d[2 * P:3 * P, :])
        nc.scalar.dma_start(out=m_t[:, 3, :], in_=m_d[3 * P:4 * P, :])
        for c in range(N_DC):
            nc.sync.dma_start(out=xT[:, c, 0:512],
                              in_=xT_d[c * P:(c + 1) * P, 0:512])
        for c in range(N_DC):
            nc.gpsimd.dma_start(out=wv_t[:, c, :], in_=wv_d[c * P:(c + 1) * P, :])
        # raw x in 8-keytile pieces so early key tiles land before PV starts
        KTC = 8
        for g in range(N_KT // KTC):
            ks = slice(g * KTC, (g + 1) * KTC)
            nc.gpsimd.dma_start(out=xA[:, ks, 0:256], in_=xsw_d[:, ks, 0:256])
            nc.gpsimd.dma_start(out=xB[:, ks, :], in_=xsw_d[:, ks, 256:512])
        for c in range(N_DC):
            nc.scalar.dma_start(out=xT[:, c, 512:1024],
                                in_=xT_d[c * P:(c + 1) * P, 512:1024])
        for c, eng in ((0, nc.sync), (1, nc.sync), (2, nc.scalar),
                       (3, nc.scalar)):
            eng.dma_start(out=xT[:, c, 1024:S],
                          in_=xT_d[c * P:(c + 1) * P, 1024:S])

        # ---- query transform qt = M^T x_q -------------------------------
        qT = big.tile([P, N_DC, QLEN], F16, tag="qT")
        for qh in range(QLEN // 512):
            for do in range(N_DC):
                pp = ps_a.tile([P, 512], F32, tag="a", name="pp")
                for c in range(N_DC):
                    nc.tensor.matmul(
                        pp[:], m_t[:, c, do * P:(do + 1) * P],
                        xT[:, c, qh * 512:(qh + 1) * 512],
                        start=(c == 0), stop=(c == N_DC - 1))
                nc.vector.tensor_copy(qT[:, do, qh * 512:(qh + 1) * 512], pp[:])

        # ---- attention --------------------------------------------------
        for qb in range(N_QB):
            q0 = qb * QB
            cxA = ps_cx.tile([P, 257], F32, tag="cxA", name="cxA")
            cxB = ps_cx.tile([P, 257], F32, tag="cxB", name="cxB")
            cxP0 = ps_cx.tile([P, 256], F32, tag="cxP0", name="cxP0")
            cxP1 = ps_cx.tile([P, 256], F32, tag="cxP1", name="cxP1")
            probs = {}

            def emit_sc(kt):
                psc = ps_a.tile([P, QB], F32, tag="a", name="psc")
                for c in range(N_DC):
                    nc.tensor.matmul(
                        psc[:], xT[:, c, kt * P:(kt + 1) * P],
                        qT[:, c, q0:q0 + QB],
                        start=(c == 0), stop=(c == N_DC - 1))
                pr = pr_pool.tile([P, QB], F16, tag="pr", name="pr")
                nc.scalar.activation(pr[:], psc[:], AF.Exp)
                probs[kt] = pr

            def emit_pv(kt):
                pr = probs.pop(kt)
                first = kt == 0
                last = kt == N_KT - 1
                nc.tensor.matmul(cxA[:], pr[:, 0:P], xA[:, kt, :],
                                 start=first, stop=last)
                nc.tensor.matmul(cxP0[:], pr[:, 0:P], xB[:, kt, :],
                                 start=first, stop=last)
                nc.tensor.matmul(cxB[:], pr[:, P:QB], xA[:, kt, :],
                                 start=first, stop=last)
                nc.tensor.matmul(cxP1[:], pr[:, P:QB], xB[:, kt, :],
                                 start=first, stop=last)

            emit_sc(0)
            emit_sc(1)
            for kt in range(N_KT):
                if kt + 2 < N_KT:
                    emit_sc(kt + 2)
                emit_pv(kt)

            # ---- flush: recip, ct copies, transpose, Wv apply, store ----
            recipA = st_pool.tile([P, 1], F32, tag="recipA", name="recipA")
            nc.vector.reciprocal(recipA[:], cxA[:, 256:257])
            recipB = st_pool.tile([P, 1], F32, tag="recipB", name="recipB")
            nc.vector.reciprocal(recipB[:], cxB[:, 256:257])
            # ct[qsub] = [P, 512] fp16 unnormalized P^T X for 128 queries;
            # qsub0 flush runs on DVE, qsub1 on ACT, in parallel.
            ct0 = ct_pool.tile([P, D], F16, tag="ct0", name="ct0")
            ct1 = ct_pool.tile([P, D], F16, tag="ct1", name="ct1")
            nc.vector.tensor_copy(ct0[:, 0:256], cxA[:, 0:256])
            nc.scalar.copy(ct1[:, 0:256], cxB[:, 0:256])
            nc.vector.tensor_copy(ct0[:, 256:512], cxP0[:])
            nc.scalar.copy(ct1[:, 256:512], cxP1[:])
            for h, (ct, recip, st_eng) in enumerate(
                    ((ct0, recipA, nc.sync), (ct1, recipB, nc.gpsimd))):
                trp = ps_tr.tile([P, N_DC, P], F16, tag="tr", name="trp")
                for c in range(N_DC):
                    nc.tensor.matmul(trp[:, c, :], ct[:, c * P:(c + 1) * P],
                                     ident[:], is_transpose=True,
                                     start=True, stop=True)
                ctT = ct_pool.tile([P, N_DC, P], F16, tag=f"ctT{h}",
                                   name="ctT")
                if h == 0:
                    nc.vector.tensor_copy(ctT[:], trp[:])
                else:
                    nc.scalar.copy(ctT[:], trp[:])
                osb = st_pool.tile([P, D], F32, tag=f"osb{h}", name="osb")
                for half in range(2):
                    po = ps_a.tile([P, 256], F32, tag="a", name="po")
                    for c in range(N_DC):
                        nc.tensor.matmul(
                            po[:], ctT[:, c, :],
                            wv_t[:, c, half * 256:(half + 1) * 256],
                            start=(c == 0), stop=(c == N_DC - 1))
                    if h == 0:
                        nc.vector.tensor_scalar_mul(
                            osb[:, half * 256:(half + 1) * 256], po[:],
                            recip[:])
                    else:
                        nc.scalar.mul(
                            osb[:, half * 256:(half + 1) * 256], po[:],
                            recip[:])
                qrow = q0 + h * P
                for half in range(2):
                    cols = slice(half * 256, (half + 1) * 256)
                    st_eng.dma_start(out=out_d[qrow:qrow + P, cols],
                                     in_=osb[:, cols])

    nc.compile()
    return nc


_CACHE = {}


def _get_nc(nreps=1):
    if nreps not in _CACHE:
        _CACHE[nreps] = _build(nreps)
    return _CACHE[nreps]


def _in_maps(x, W_query, W_key, W_value):
    x = np.asarray(x, dtype=np.float32)
    wq64 = np.asarray(W_query, dtype=np.float64)
    wk64 = np.asarray(W_key, dtype=np.float64)
    m = np.ascontiguousarray((wq64 @ wk64.T) * SCALE).astype(np.float16)
    wv = np.ascontiguousarray(np.asarray(W_value, dtype=np.float32)).astype(
        np.float16)
    maps = []
    for core in range(N_CORES):
        b = core // CORES_PER_B
        q0 = (core % CORES_PER_B) * QLEN
        xr = np.roll(x[b], -q0, axis=0)
        xT = np.ascontiguousarray(xr.T.astype(np.float16))
        xsw = np.ascontiguousarray(
            xr.astype(np.float16).reshape(N_KT, P, D).transpose(1, 0, 2))
        maps.append({"xT": xT, "xsw": xsw, "m": m, "wv": wv})
    return maps


def kernel(x, W_query, W_key, W_value, _trace=False):
    import os
    if not _trace:
        # NTFF tracing is unavailable here; make sure an inherited
        # BASS_TRACE can't route execution down that path.
        os.environ.setdefault("BASS_NEVER_TRACE", "1")
    nc = _get_nc()
    maps = _in_maps(x, W_query, W_key, W_value)
    res = run_bass_kernel_spmd(nc, maps, list(range(N_CORES)), trace=_trace)
    out = np.empty((B, S, D), dtype=np.float32)
    for core in range(N_CORES):
        b = core // CORES_PER_B
        q0 = (core % CORES_PER_B) * QLEN
        out[b, q0:q0 + QLEN] = res.results[core]["out"]
    if _trace:
        return out, res
    return out


# revision 12
# speedup vs baseline: 1.6226x; 1.0476x over previous
"""Full-attention kernel (QKV projections + softmax(QK^T/sqrt(d))V) on 8
trn2 NeuronCores.

Problem: x [2,4096,512] f32, W_{q,k,v} [512,512] f32 -> context [2,4096,512]
f32 (the reference applies no causal mask and dropout=0).

Distribution (data parallel, no collectives): core c handles batch b = c // 4
and query block q0 = (c % 4) * 1024; the host rotates each core's copy of
x[b] so its query rows come first (attention is permutation-invariant over
keys) and ships both x and x^T in fp16.

Algebraic restructure (the big lever): per core, queries (1024) are 4x fewer
than keys (4096), so every weight application is folded onto the query side:
  * scores   s = x_q^T (Wq Wk^T / sqrt(d)) x_k = qt . x_k  with
    qt = M^T x_q, M = (Wq Wk^T) / sqrt(d) precomputed on the host in f64.
    -> NO key projection; x^T itself is the transposed-key matmul operand.
  * context  ctx = P^T X Wv = (P^T X) Wv: accumulate ct = P^T X against raw
    x, then apply Wv once per 128-query block. -> NO value projection.
PE work per core: qt 16.4k + scores 131k + P^T X 131.3k + ct transposes
4.1k + Wv apply 16.4k ~= 300k cycles (vs 483k direct / 410k with separate
K,V projections).

Layout/precision:
  * scores are computed TRANSPOSED, [k_tile=128, q=256]: lhsT = x^T chunks,
    rhs = qt chunks; exp(scores^T) on ACT feeds the P^T X matmul directly
    as stationary -- no probability transposes.
  * the softmax row-sum is folded into the P^T X matmul via a ones-column
    appended to raw x (257-col tiles): each query's sum(exp) accumulates in
    psum column 256, landing per-partition exactly where the 1/rowsum
    normalization needs it (cost: 1 extra psum column per matmul).
  * fp16 operands everywhere (1 cyc/row on PE), f32 psum accumulation,
    f32 output.
"""
import numpy as np
from contextlib import ExitStack

from concourse import bacc
import concourse.mybir as mybir
import concourse.tile as tile
from concourse.bass_utils import run_bass_kernel_spmd
from concourse.masks import make_identity

F32 = mybir.dt.float32
F32R = mybir.dt.float32r
F16 = mybir.dt.float16
AF = mybir.ActivationFunctionType

B, S, D = 2, 4096, 512
N_CORES = 8
CORES_PER_B = N_CORES // B
QLEN = S // CORES_PER_B             # 1024
P = 128
SCALE = 1.0 / float(np.sqrt(D))

N_DC = D // P                       # 4 contraction chunks
N_KT = S // P                       # 32 key tiles
QB = 256                            # query block (psum-bank friendly)
N_QB = QLEN // QB                   # 4


def _build(nreps=1):
    nc = bacc.Bacc(None)
    xT_d = nc.declare_dram_parameter("xT", [D, S], F16, isOutput=False)
    # raw x pre-swizzled on host to [P, N_KT, D]: xsw[p, kt, :] = x[kt*128+p]
    xsw_d = nc.declare_dram_parameter("xsw", [P, N_KT, D], F16, isOutput=False)
    m_d = nc.declare_dram_parameter("m", [D, D], F16, isOutput=False)
    wv_d = nc.declare_dram_parameter("wv", [D, D], F16, isOutput=False)
    out_d = nc.declare_dram_parameter("out", [QLEN, D], F32, isOutput=True)

    with tile.TileContext(nc) as tc, ExitStack() as ctx:
        const = ctx.enter_context(tc.tile_pool(name="const", bufs=1))
        big = ctx.enter_context(tc.tile_pool(name="big", bufs=1))
        pr_pool = ctx.enter_context(tc.tile_pool(name="pr", bufs=4))
        st_pool = ctx.enter_context(tc.tile_pool(name="st", bufs=2))
        ct_pool = ctx.enter_context(tc.tile_pool(name="ct", bufs=2))
        # psum budget (8 banks): "a" ring 3 + cx (2+1+1) + tr 1
        ps_a = ctx.enter_context(tc.tile_pool(name="ps_a", bufs=3, space="PSUM"))
        ps_cx = ctx.enter_context(tc.tile_pool(name="ps_cx", bufs=1, space="PSUM"))
        ps_tr = ctx.enter_context(tc.tile_pool(name="ps_tr", bufs=1, space="PSUM"))

        ident_f = const.tile([P, P], F32)
        make_identity(nc, ident_f[:])
        ident = const.tile([P, P], F16)
        nc.vector.tensor_copy(ident[:], ident_f[:])

        # ---- input DMAs -------------------------------------------------
        m_t = big.tile([P, N_DC, D], F16, tag="m")
        wv_t = big.tile([P, N_DC, D], F16, tag="wv")
        xT = big.tile([P, N_DC, S], F16, tag="xT")
        xA = big.tile([P, N_KT, 257], F16, tag="xA")
        xB = big.tile([P, N_KT, 256], F16, tag="xB")
        nc.vector.memset(xA[:, :, 256:257], 1.0)

        # queue plan: sync/scalar carry M then xT (query cols first);
        # gpsimd carries wv then the two big raw-x swizzled loads.
        nc.sync.dma_start(out=m_t[:, 0, :], in_=m_d[0:P, :])
        nc.sync.dma_start(out=m_t[:, 1, :], in_=m_d[P:2 * P, :])
        nc.scalar.dma_start(out=m_t[:, 2, :], in_=m_d[2 * P:3 * P, :])
        nc.scalar.dma_start(out=m_t[:, 3, :], in_=m_d[3 * P:4 * P, :])
        for c in range(N_DC):
            nc.sync.dma_start(out=xT[:, c, 0:512],
                              in_=xT_d[c * P:(c + 1) * P, 0:512])
        for c in range(N_DC):
            nc.gpsimd.dma_start(out=wv_t[:, c, :], in_=wv_d[c * P:(c + 1) * P, :])
        # raw x in 8-keytile pieces so early key tiles land before PV starts
        KTC = 8
        for g in range(N_KT // KTC):
            ks = slice(g * KTC, (g + 1) * KTC)
            nc.gpsimd.dma_start(out=xA[:, ks, 0:256], in_=xsw_d[:, ks, 0:256])
            nc.gpsimd.dma_start(out=xB[:, ks, :], in_=xsw_d[:, ks, 256:512])
        for c in range(N_DC):
            nc.scalar.dma_start(out=xT[:, c, 512:1024],
                                in_=xT_d[c * P:(c + 1) * P, 512:1024])
        for c, eng in ((0, nc.sync), (1, nc.sync), (2, nc.scalar),
                       (3, nc.scalar)):
            eng.dma_start(out=xT[:, c, 1024:S],
                          in_=xT_d[c * P:(c + 1) * P, 1024:S])

        # ---- query transform qt = M^T x_q -------------------------------
        qT = big.tile([P, N_DC, QLEN], F16, tag="qT")
        for qh in range(QLEN // 512):
            for do in range(N_DC):
                pp = ps_a.tile([P, 512], F32, tag="a", name="pp")
                for c in range(N_DC):
                    nc.tensor.matmul(
                        pp[:], m_t[:, c, do * P:(do + 1) * P],
                        xT[:, c, qh * 512:(qh + 1) * 512],
                        start=(c == 0), stop=(c == N_DC - 1))
                nc.vector.tensor_copy(qT[:, do, qh * 512:(qh + 1) * 512], pp[:])

        # ---- attention --------------------------------------------------
        # The per-block flush (recip, ct copies, transpose, Wv apply, store)
        # is software-pipelined across block boundaries: its PE ops are
        # emitted as closures interleaved into the NEXT block's score/PV
        # stream, so the PE never idles waiting on the DVE/ACT copy chain.
        def make_flush(cxA, cxB, cxP0, cxP1, q0):
            recipA = st_pool.tile([P, 1], F32, tag="recipA", name="recipA")
            nc.vector.reciprocal(recipA[:], cxA[:, 256:257])
            recipB = st_pool.tile([P, 1], F32, tag="recipB", name="recipB")
            nc.vector.reciprocal(recipB[:], cxB[:, 256:257])
            ct0 = ct_pool.tile([P, D], F16, tag="ct0", name="ct0")
            ct1 = ct_pool.tile([P, D], F16, tag="ct1", name="ct1")
            nc.vector.tensor_copy(ct0[:, 0:256], cxA[:, 0:256])
            nc.scalar.copy(ct1[:, 0:256], cxB[:, 0:256])
            nc.vector.tensor_copy(ct0[:, 256:512], cxP0[:])
            nc.scalar.copy(ct1[:, 256:512], cxP1[:])
            cts = (ct0, ct1)
            recips = (recipA, recipB)
            ctTs = {}
            osbs = {}

            def tr(h):
                def f():
                    trp = ps_tr.tile([P, N_DC, P], F16, tag="tr", name="trp")
                    for c in range(N_DC):
                        nc.tensor.matmul(trp[:, c, :],
                                         cts[h][:, c * P:(c + 1) * P],
                                         ident[:], is_transpose=True,
                                         start=True, stop=True)
                    ctT = ct_pool.tile([P, N_DC, P], F16, tag=f"ctT{h}",
                                       name="ctT")
                    if h == 0:
                        nc.vector.tensor_copy(ctT[:], trp[:])
                    else:
                        nc.scalar.copy(ctT[:], trp[:])
                    ctTs[h] = ctT
                return f

            def po(h, half):
                def f():
                    if half == 0:
                        osbs[h] = st_pool.tile([P, D], F32, tag=f"osb{h}",
                                               name="osb")
                    osb = osbs[h]
                    p = ps_a.tile([P, 256], F32, tag="a", name="po")
                    for c in range(N_DC):
                        nc.tensor.matmul(
                            p[:], ctTs[h][:, c, :],
                            wv_t[:, c, half * 256:(half + 1) * 256],
                            start=(c == 0), stop=(c == N_DC - 1))
                    cols = slice(half * 256, (half + 1) * 256)
                    if h == 0:
                        nc.vector.tensor_scalar_mul(osb[:, cols], p[:],
                                                    recips[h][:])
                    else:
                        nc.scalar.mul(osb[:, cols], p[:], recips[h][:])
                    qrow = q0 + h * P
                    st_eng = nc.sync if h == 0 else nc.gpsimd
                    st_eng.dma_start(out=out_d[qrow:qrow + P, cols],
                                     in_=osb[:, cols])
                return f

            return [tr(0), tr(1), po(0, 0), po(0, 1), po(1, 0), po(1, 1)]

        pending = []
        for qb in range(N_QB):
            q0 = qb * QB
            cxA = ps_cx.tile([P, 257], F32, tag="cxA", name="cxA")
            cxB = ps_cx.tile([P, 257], F32, tag="cxB", name="cxB")
            cxP0 = ps_cx.tile([P, 256], F32, tag="cxP0", name="cxP0")
            cxP1 = ps_cx.tile([P, 256], F32, tag="cxP1", name="cxP1")
            probs = {}

            def emit_sc(kt):
                psc = ps_a.tile([P, QB], F32, tag="a", name="psc")
                for c in range(N_DC):
                    nc.tensor.matmul(
                        psc[:], xT[:, c, kt * P:(kt + 1) * P],
                        qT[:, c, q0:q0 + QB],
                        start=(c == 0), stop=(c == N_DC - 1))
                pr = pr_pool.tile([P, QB], F16, tag="pr", name="pr")
                nc.scalar.activation(pr[:], psc[:], AF.Exp)
                probs[kt] = pr

            def emit_pv(kt):
                pr = probs.pop(kt)
                first = kt == 0
                last = kt == N_KT - 1
                nc.tensor.matmul(cxA[:], pr[:, 0:P], xA[:, kt, :],
                                 start=first, stop=last)
                nc.tensor.matmul(cxP0[:], pr[:, 0:P], xB[:, kt, :],
                                 start=first, stop=last)
                nc.tensor.matmul(cxB[:], pr[:, P:QB], xA[:, kt, :],
                                 start=first, stop=last)
                nc.tensor.matmul(cxP1[:], pr[:, P:QB], xB[:, kt, :],
                                 start=first, stop=last)

            emit_sc(0)
            emit_sc(1)
            for kt in range(N_KT):
                if kt + 2 < N_KT:
                    emit_sc(kt + 2)
                emit_pv(kt)
                if pending:
                    pending.pop(0)()
            pending = make_flush(cxA, cxB, cxP0, cxP1, q0)
        for f in pending:
            f()

    nc.compile()
    return nc


_CACHE = {}


def _get_nc(nreps=1):
    if nreps not in _CACHE:
        _CACHE[nreps] = _build(nreps)
    return _CACHE[nreps]


def _in_maps(x, W_query, W_key, W_value):
    x = np.asarray(x, dtype=np.float32)
    wq64 = np.asarray(W_query, dtype=np.float64)
    wk64 = np.asarray(W_key, dtype=np.float64)
    m = np.ascontiguousarray((wq64 @ wk64.T) * SCALE).astype(np.float16)
    wv = np.ascontiguousarray(np.asarray(W_value, dtype=np.float32)).astype(
        np.float16)
    maps = []
    for core in range(N_CORES):
        b = core // CORES_PER_B
        q0 = (core % CORES_PER_B) * QLEN
        xr = np.roll(x[b], -q0, axis=0)
        xT = np.ascontiguousarray(xr.T.astype(np.float16))
        xsw = np.ascontiguousarray(
            xr.astype(np.float16).reshape(N_KT, P, D).transpose(1, 0, 2))
        maps.append({"xT": xT, "xsw": xsw, "m": m, "wv": wv})
    return maps


def kernel(x, W_query, W_key, W_value, _trace=False):
    import os
    if not _trace:
        # NTFF tracing is unavailable here; make sure an inherited
        # BASS_TRACE can't route execution down that path.
        os.environ.setdefault("BASS_NEVER_TRACE", "1")
    nc = _get_nc()
    maps = _in_maps(x, W_query, W_key, W_value)
    res = run_bass_kernel_spmd(nc, maps, list(range(N_CORES)), trace=_trace)
    out = np.empty((B, S, D), dtype=np.float32)
    for core in range(N_CORES):
        b = core // CORES_PER_B
        q0 = (core % CORES_PER_B) * QLEN
        out[b, q0:q0 + QLEN] = res.results[core]["out"]
    if _trace:
        return out, res
    return out


# revision 13
# speedup vs baseline: 1.6253x; 1.0017x over previous
"""Full-attention kernel (QKV projections + softmax(QK^T/sqrt(d))V) on 8
trn2 NeuronCores.

Problem: x [2,4096,512] f32, W_{q,k,v} [512,512] f32 -> context [2,4096,512]
f32 (the reference applies no causal mask and dropout=0).

Distribution (data parallel, no collectives): core c handles batch b = c // 4
and query block q0 = (c % 4) * 1024; the host rotates each core's copy of
x[b] so its query rows come first (attention is permutation-invariant over
keys) and ships both x and x^T in fp16.

Algebraic restructure (the big lever): per core, queries (1024) are 4x fewer
than keys (4096), so every weight application is folded onto the query side:
  * scores   s = x_q^T (Wq Wk^T / sqrt(d)) x_k = qt . x_k  with
    qt = M^T x_q, M = (Wq Wk^T) / sqrt(d) precomputed on the host in f64.
    -> NO key projection; x^T itself is the transposed-key matmul operand.
  * context  ctx = P^T X Wv = (P^T X) Wv: accumulate ct = P^T X against raw
    x, then apply Wv once per 128-query block. -> NO value projection.
PE work per core: qt 16.4k + scores 131k + P^T X 131.3k + ct transposes
4.1k + Wv apply 16.4k ~= 300k cycles (vs 483k direct / 410k with separate
K,V projections).

Layout/precision:
  * scores are computed TRANSPOSED, [k_tile=128, q=256]: lhsT = x^T chunks,
    rhs = qt chunks; exp(scores^T) on ACT feeds the P^T X matmul directly
    as stationary -- no probability transposes.
  * the softmax row-sum is folded into the P^T X matmul via a ones-column
    appended to raw x (257-col tiles): each query's sum(exp) accumulates in
    psum column 256, landing per-partition exactly where the 1/rowsum
    normalization needs it (cost: 1 extra psum column per matmul).
  * fp16 operands everywhere (1 cyc/row on PE), f32 psum accumulation,
    f32 output.
"""
import numpy as np
from contextlib import ExitStack

from concourse import bacc
import concourse.mybir as mybir
import concourse.tile as tile
from concourse.bass_utils import run_bass_kernel_spmd
from concourse.masks import make_identity

F32 = mybir.dt.float32
F32R = mybir.dt.float32r
F16 = mybir.dt.float16
AF = mybir.ActivationFunctionType

B, S, D = 2, 4096, 512
N_CORES = 8
CORES_PER_B = N_CORES // B
QLEN = S // CORES_PER_B             # 1024
P = 128
SCALE = 1.0 / float(np.sqrt(D))

N_DC = D // P                       # 4 contraction chunks
N_KT = S // P                       # 32 key tiles
QB = 256                            # query block (psum-bank friendly)
N_QB = QLEN // QB                   # 4


def _build(nreps=1):
    nc = bacc.Bacc(None)
    xT_d = nc.declare_dram_parameter("xT", [D, S], F16, isOutput=False)
    # raw x pre-swizzled on host to [P, N_KT, D]: xsw[p, kt, :] = x[kt*128+p]
    xsw_d = nc.declare_dram_parameter("xsw", [P, N_KT, D], F16, isOutput=False)
    m_d = nc.declare_dram_parameter("m", [D, D], F16, isOutput=False)
    wv_d = nc.declare_dram_parameter("wv", [D, D], F16, isOutput=False)
    out_d = nc.declare_dram_parameter("out", [QLEN, D], F32, isOutput=True)

    with tile.TileContext(nc) as tc, ExitStack() as ctx:
        const = ctx.enter_context(tc.tile_pool(name="const", bufs=1))
        big = ctx.enter_context(tc.tile_pool(name="big", bufs=1))
        pr_pool = ctx.enter_context(tc.tile_pool(name="pr", bufs=4))
        st_pool = ctx.enter_context(tc.tile_pool(name="st", bufs=2))
        ct_pool = ctx.enter_context(tc.tile_pool(name="ct", bufs=2))
        # psum budget (8 banks): "a" ring 3 + cx (2+1+1) + tr 1
        ps_a = ctx.enter_context(tc.tile_pool(name="ps_a", bufs=3, space="PSUM"))
        ps_cx = ctx.enter_context(tc.tile_pool(name="ps_cx", bufs=1, space="PSUM"))
        ps_tr = ctx.enter_context(tc.tile_pool(name="ps_tr", bufs=1, space="PSUM"))

        ident_f = const.tile([P, P], F32)
        make_identity(nc, ident_f[:])
        ident = const.tile([P, P], F16)
        nc.vector.tensor_copy(ident[:], ident_f[:])

        # ---- input DMAs -------------------------------------------------
        m_t = big.tile([P, N_DC, D], F16, tag="m")
        wv_t = big.tile([P, N_DC, D], F16, tag="wv")
        xT = big.tile([P, N_DC, S], F16, tag="xT")
        xA = big.tile([P, N_KT, 257], F16, tag="xA")
        xB = big.tile([P, N_KT, 256], F16, tag="xB")
        nc.vector.memset(xA[:, :, 256:257], 1.0)

        # queue plan: sync/scalar carry M then xT (query cols first);
        # gpsimd carries wv then the two big raw-x swizzled loads.
        nc.sync.dma_start(out=m_t[:, 0, :], in_=m_d[0:P, :])
        nc.scalar.dma_start(out=m_t[:, 1, :], in_=m_d[P:2 * P, :])
        nc.gpsimd.dma_start(out=m_t[:, 2, :], in_=m_d[2 * P:3 * P, :])
        nc.gpsimd.dma_start(out=m_t[:, 3, :], in_=m_d[3 * P:4 * P, :])
        nc.sync.dma_start(out=xT[:, 0, 0:512], in_=xT_d[0:P, 0:512])
        nc.scalar.dma_start(out=xT[:, 1, 0:512], in_=xT_d[P:2 * P, 0:512])
        nc.gpsimd.dma_start(out=xT[:, 2, 0:512], in_=xT_d[2 * P:3 * P, 0:512])
        nc.sync.dma_start(out=xT[:, 3, 0:512], in_=xT_d[3 * P:4 * P, 0:512])
        for c in range(N_DC):
            nc.gpsimd.dma_start(out=wv_t[:, c, :], in_=wv_d[c * P:(c + 1) * P, :])
        # raw x in 8-keytile pieces so early key tiles land before PV starts
        KTC = 8
        for g in range(N_KT // KTC):
            ks = slice(g * KTC, (g + 1) * KTC)
            nc.gpsimd.dma_start(out=xA[:, ks, 0:256], in_=xsw_d[:, ks, 0:256])
            nc.gpsimd.dma_start(out=xB[:, ks, :], in_=xsw_d[:, ks, 256:512])
        for c in range(N_DC):
            nc.scalar.dma_start(out=xT[:, c, 512:1024],
                                in_=xT_d[c * P:(c + 1) * P, 512:1024])
        for c, eng in ((0, nc.sync), (1, nc.sync), (2, nc.scalar),
                       (3, nc.scalar)):
            eng.dma_start(out=xT[:, c, 1024:S],
                          in_=xT_d[c * P:(c + 1) * P, 1024:S])

        # ---- query transform qt = M^T x_q -------------------------------
        qT = big.tile([P, N_DC, QLEN], F16, tag="qT")
        for qh in range(QLEN // 512):
            for do in range(N_DC):
                pp = ps_a.tile([P, 512], F32, tag="a", name="pp")
                for c in range(N_DC):
                    nc.tensor.matmul(
                        pp[:], m_t[:, c, do * P:(do + 1) * P],
                        xT[:, c, qh * 512:(qh + 1) * 512],
                        start=(c == 0), stop=(c == N_DC - 1))
                nc.vector.tensor_copy(qT[:, do, qh * 512:(qh + 1) * 512], pp[:])

        # ---- attention --------------------------------------------------
        # The per-block flush (recip, ct copies, transpose, Wv apply, store)
        # is software-pipelined across block boundaries: its PE ops are
        # emitted as closures interleaved into the NEXT block's score/PV
        # stream, so the PE never idles waiting on the DVE/ACT copy chain.
        def make_flush(cxA, cxB, cxP0, cxP1, q0):
            recipA = st_pool.tile([P, 1], F32, tag="recipA", name="recipA")
            nc.vector.reciprocal(recipA[:], cxA[:, 256:257])
            recipB = st_pool.tile([P, 1], F32, tag="recipB", name="recipB")
            nc.vector.reciprocal(recipB[:], cxB[:, 256:257])
            ct0 = ct_pool.tile([P, D], F16, tag="ct0", name="ct0")
            ct1 = ct_pool.tile([P, D], F16, tag="ct1", name="ct1")
            nc.vector.tensor_copy(ct0[:, 0:256], cxA[:, 0:256])
            nc.scalar.copy(ct1[:, 0:256], cxB[:, 0:256])
            nc.vector.tensor_copy(ct0[:, 256:512], cxP0[:])
            nc.scalar.copy(ct1[:, 256:512], cxP1[:])
            cts = (ct0, ct1)
            recips = (recipA, recipB)
            ctTs = {}
            osbs = {}

            def tr(h):
                def f():
                    trp = ps_tr.tile([P, N_DC, P], F16, tag="tr", name="trp")
                    for c in range(N_DC):
                        nc.tensor.matmul(trp[:, c, :],
                                         cts[h][:, c * P:(c + 1) * P],
                                         ident[:], is_transpose=True,
                                         start=True, stop=True)
                    ctT = ct_pool.tile([P, N_DC, P], F16, tag=f"ctT{h}",
                                       name="ctT")
                    if h == 0:
                        nc.vector.tensor_copy(ctT[:], trp[:])
                    else:
                        nc.scalar.copy(ctT[:], trp[:])
                    ctTs[h] = ctT
                return f

            def po(h, half):
                def f():
                    if half == 0:
                        osbs[h] = st_pool.tile([P, D], F32, tag=f"osb{h}",
                                               name="osb")
                    osb = osbs[h]
                    p = ps_a.tile([P, 256], F32, tag="a", name="po")
                    for c in range(N_DC):
                        nc.tensor.matmul(
                            p[:], ctTs[h][:, c, :],
                            wv_t[:, c, half * 256:(half + 1) * 256],
                            start=(c == 0), stop=(c == N_DC - 1))
                    cols = slice(half * 256, (half + 1) * 256)
                    if h == 0:
                        nc.vector.tensor_scalar_mul(osb[:, cols], p[:],
                                                    recips[h][:])
                    else:
                        nc.scalar.mul(osb[:, cols], p[:], recips[h][:])
                    qrow = q0 + h * P
                    st_eng = nc.sync if h == 0 else nc.gpsimd
                    st_eng.dma_start(out=out_d[qrow:qrow + P, cols],
                                     in_=osb[:, cols])
                return f

            return [tr(0), tr(1), po(0, 0), po(0, 1), po(1, 0), po(1, 1)]

        pending = []
        for qb in range(N_QB):
            q0 = qb * QB
            cxA = ps_cx.tile([P, 257], F32, tag="cxA", name="cxA")
            cxB = ps_cx.tile([P, 257], F32, tag="cxB", name="cxB")
            cxP0 = ps_cx.tile([P, 256], F32, tag="cxP0", name="cxP0")
            cxP1 = ps_cx.tile([P, 256], F32, tag="cxP1", name="cxP1")
            probs = {}

            def emit_sc(kt):
                psc = ps_a.tile([P, QB], F32, tag="a", name="psc")
                for c in range(N_DC):
                    nc.tensor.matmul(
                        psc[:], xT[:, c, kt * P:(kt + 1) * P],
                        qT[:, c, q0:q0 + QB],
                        start=(c == 0), stop=(c == N_DC - 1))
                pr = pr_pool.tile([P, QB], F16, tag="pr", name="pr")
                nc.scalar.activation(pr[:], psc[:], AF.Exp)
                probs[kt] = pr

            def emit_pv(kt):
                pr = probs.pop(kt)
                first = kt == 0
                last = kt == N_KT - 1
                nc.tensor.matmul(cxA[:], pr[:, 0:P], xA[:, kt, :],
                                 start=first, stop=last)
                nc.tensor.matmul(cxP0[:], pr[:, 0:P], xB[:, kt, :],
                                 start=first, stop=last)
                nc.tensor.matmul(cxB[:], pr[:, P:QB], xA[:, kt, :],
                                 start=first, stop=last)
                nc.tensor.matmul(cxP1[:], pr[:, P:QB], xB[:, kt, :],
                                 start=first, stop=last)

            emit_sc(0)
            emit_sc(1)
            for kt in range(N_KT):
                if kt + 2 < N_KT:
                    emit_sc(kt + 2)
                emit_pv(kt)
                if pending:
                    pending.pop(0)()
            pending = make_flush(cxA, cxB, cxP0, cxP1, q0)
        for f in pending:
            f()

    nc.compile()
    return nc


_CACHE = {}


def _get_nc(nreps=1):
    if nreps not in _CACHE:
        _CACHE[nreps] = _build(nreps)
    return _CACHE[nreps]


def _in_maps(x, W_query, W_key, W_value):
    x = np.asarray(x, dtype=np.float32)
    wq64 = np.asarray(W_query, dtype=np.float64)
    wk64 = np.asarray(W_key, dtype=np.float64)
    m = np.ascontiguousarray((wq64 @ wk64.T) * SCALE).astype(np.float16)
    wv = np.ascontiguousarray(np.asarray(W_value, dtype=np.float32)).astype(
        np.float16)
    maps = []
    for core in range(N_CORES):
        b = core // CORES_PER_B
        q0 = (core % CORES_PER_B) * QLEN
        xr = np.roll(x[b], -q0, axis=0)
        xT = np.ascontiguousarray(xr.T.astype(np.float16))
        xsw = np.ascontiguousarray(
            xr.astype(np.float16).reshape(N_KT, P, D).transpose(1, 0, 2))
        maps.append({"xT": xT, "xsw": xsw, "m": m, "wv": wv})
    return maps


def kernel(x, W_query, W_key, W_value, _trace=False):
    import os
    if not _trace:
        # NTFF tracing is unavailable here; make sure an inherited
        # BASS_TRACE can't route execution down that path.
        os.environ.setdefault("BASS_NEVER_TRACE", "1")
    nc = _get_nc()
    maps = _in_maps(x, W_query, W_key, W_value)
    res = run_bass_kernel_spmd(nc, maps, list(range(N_CORES)), trace=_trace)
    out = np.empty((B, S, D), dtype=np.float32)
    for core in range(N_CORES):
        b = core // CORES_PER_B
        q0 = (core % CORES_PER_B) * QLEN
        out[b, q0:q0 + QLEN] = res.results[core]["out"]
    if _trace:
        return out, res
    return out
